# revision 22
# baseline (speedup 1.0000x reference)
"""Trainium2 Bass kernel for the pre-norm attention + SwiGLU FFN layer (v2).

Sharding: tokens (batch*seq flattened) split across 8 cores - 512 tokens
each; cores 0-3 hold batch 0, cores 4-7 batch 1. All per-token work (LNs,
projections, rope, FFN) is local with replicated weights; attention gathers
rope'd K (bf16) and ones-padded V (fp8e4m3) across each 4-core batch group
with two AllGathers (K first so scores can start while V is in flight),
then each core attends its 512 queries over the full 2048 context.

v2 vs v1:
 - all matmuls in bf16 (same PE rate as f32r, half the bytes); PV matmul in
   fp8 DoubleRow (2x PE rate; exp(scores) stays in [e^-5, e^4.6], inside
   e4m3 range, ones column exact).
 - transposes done by the DMA XBAR (dma_start_transpose, 2-byte dtype)
   instead of PE transposes + vector copy-backs.
 - weights are host-side pre-tiled so every weight DMA is contiguous per
   partition (2-16 KB lines instead of the 512 B packet storm).
 - denominators: ones-row PV output broadcast via one sel-matmul, one
   vector reciprocal on 64 partitions, one multiply (vs 32 single-partition
   reciprocals).
 - K/V projection and rope run before Q's so the collectives trigger early;
   Q-side prep overlaps the gathers.
 - attention inner loop is software-pipelined (scores for pair p+1 issued
   before the PV of pair p) to keep the PE p-state up.
"""

import numpy as np
import ml_dtypes

import bass_rust
import concourse.bass as bass
import concourse.mybir as mybir
import concourse.tile as tile
from concourse.bass_utils import run_bass_kernel_spmd
from concourse.vector_clock import ScopedClock

F32 = mybir.dt.float32
F32R = mybir.dt.float32r
BF16 = mybir.dt.bfloat16
FP8 = mybir.dt.float8e4
AF = mybir.ActivationFunctionType
DR = mybir.MatmulPerfMode.DoubleRow

N_CORES = 8
GROUP = 4
EPS = 1e-6

# ---------------------------------------------------------------------------
# Workaround for this walrus build's 1-wait-per-instruction encoding limit.
# ---------------------------------------------------------------------------
_MAX_WAITS = 1
_carrier_id = [0]


def _patched_drain_and_barrier(self, tick_clock, wait_clock):
    nc = self.nc
    drain_inst = nc.sync.drain()
    wait_clock.add_sem_waits(
        drain_inst.ins, ScopedClock({None: tick_clock.global_clock})
    )
    si = drain_inst.ins.sync_info
    waits = list(si.on_wait)
    if len(waits) > _MAX_WAITS:
        drain_inst.ins.sync_info = bass_rust.SyncInfo(
            on_wait=waits[:_MAX_WAITS], on_update=list(si.on_update)
        )
        rest = waits[_MAX_WAITS:]
        while rest:
            chunk, rest = rest[:_MAX_WAITS], rest[_MAX_WAITS:]
            extra = nc.sync.drain()
            extra.ins.sync_info = bass_rust.SyncInfo(on_wait=chunk, on_update=[])

    nc.all_engine_barrier()
    assert self.sems is not None
    popped = nc._tile_sem_poison_stack.pop()
    assert popped is self._sem_poison
    nc.clear_and_free_semaphores(list(self.sems.allocated().values()))
    nc.all_engine_barrier()


tile.TileContext._drain_and_barrier = _patched_drain_and_barrier


def _split_all_waits(nc, max_waits=_MAX_WAITS):
    for fn in nc.m.functions:
        for bb in fn.blocks:
            insts = list(bb.instructions)
            out = []
            changed = False
            for inst in insts:
                si = getattr(inst, "sync_info", None)
                if si is not None and si.on_wait and len(si.on_wait) > max_waits:
                    waits = list(si.on_wait)
                    updates = list(si.on_update)
                    extra, keep = waits[:-max_waits], waits[-max_waits:]
                    while extra:
                        chunk, extra = extra[:max_waits], extra[max_waits:]
                        _carrier_id[0] += 1
                        nop = mybir.InstNoOp(name=f"I-waitcar-{_carrier_id[0]}")
                        nop.engine = inst.engine
                        nop.sync_info = bass_rust.SyncInfo(on_wait=chunk, on_update=[])
                        nc.register_instruction(nop)
                        out.append(nop)
                    inst.sync_info = bass_rust.SyncInfo(on_wait=keep, on_update=updates)
                    changed = True
                out.append(inst)
            if changed:
                bb.instructions = out


# ---------------------------------------------------------------------------
# Graph builder (one SPMD program for all 8 cores)
# ---------------------------------------------------------------------------

def build_nc(T=512, D=1024, H=16, HD=64, FFN=4096, flags=frozenset()):
    NT = T // 128            # token tiles per core (4)
    ND = D // 128            # model-dim tiles (8)
    NH = FFN // 128          # ffn hidden tiles (32)
    D3 = 3 * D
    CTX = GROUP * T          # context tokens (2048)
    NKT = CTX // 128         # context k-token tiles (16)
    NPAIR = NKT // 2         # kt pairs for DoubleRow PV (8)
    VW = H * (HD + 1)        # padded v width per token (1040)
    KF = D * T               # k floats per rank
    VF = T * VW              # v elements per rank

    nc = bass.Bass(trn_type="TRN2", num_devices=N_CORES)

    x_p = nc.declare_dram_parameter("x", [T, D], F32, isOutput=False)
    cos_p = nc.declare_dram_parameter("cosfull", [T, D], BF16, isOutput=False)
    sin_p = nc.declare_dram_parameter("sinmod", [T, D], BF16, isOutput=False)
    # pre-tiled weights (see _prep_inputs for layouts)
    wqkv_p = nc.declare_dram_parameter("wqkv_t", [6, 128, ND, 512], BF16,
                                       isOutput=False)
    wout_p = nc.declare_dram_parameter("wout_t", [2, 128, ND, 512], BF16,
                                       isOutput=False)
    w1_p = nc.declare_dram_parameter("w1_t", [NH, 128, ND, 128], BF16,
                                     isOutput=False)
    w3_p = nc.declare_dram_parameter("w3_t", [NH, 128, ND, 128], BF16,
                                     isOutput=False)
    w2_p = nc.declare_dram_parameter("w2_t", [NH // 8, 128, 8, D], BF16,
                                     isOutput=False)
    vecs = {}
    for name, size in [("ln1_g", D), ("ln1_b", D), ("qn_g", D), ("qn_b", D),
                       ("kn_g", D), ("kn_b", D), ("ln2_g", D), ("ln2_b", D),
                       ("b_qkv", D3), ("b_out", D), ("b1", FFN), ("b3", FFN),
                       ("b2", D)]:
        flag = {"ln1_g": "ln1_gb", "ln1_b": "ln1_gb", "qn_g": "qn_gb",
                "qn_b": "qn_gb", "kn_g": "kn_gb", "kn_b": "kn_gb",
                "ln2_g": "ln2_gb", "ln2_b": "ln2_gb", "b_qkv": "bqkv",
                "b_out": "bout", "b1": "b1", "b3": "b3", "b2": "b2"}[name]
        if flag in flags:
            vecs[name] = nc.declare_dram_parameter(name, [size], F32,
                                                   isOutput=False)
    out_p = nc.declare_dram_parameter("out", [T, D], F32, isOutput=True)

    KVB = 2 * KF + VF        # fp8 bytes per rank (k bf16 + v fp8)
    kv_in = nc.dram_tensor("kv_in", [KVB], FP8)
    kv_all = nc.dram_tensor("kv_all", [GROUP * KVB], FP8)

    groups = [list(range(g * GROUP, (g + 1) * GROUP))
              for g in range(N_CORES // GROUP)]

    def bcast_ap(param, width):
        return bass.AP(tensor=param.ap().tensor, offset=0,
                       ap=[[0, 128], [1, width]])

    from contextlib import ExitStack
    with tile.TileContext(nc) as tc, ExitStack() as stack:
        const = stack.enter_context(tc.tile_pool(name="const", bufs=1))
        sel = const.tile([65, 64], F32, tag="sel")
        nc.vector.memset(sel, 0.0)
        nc.vector.memset(sel[64:65, :], 1.0)
        eps_t = const.tile([128, 1], F32, tag="eps")
        nc.vector.memset(eps_t, EPS)
        cosf = const.tile([128, NT, D], BF16, tag="cosf")
        sinm = const.tile([128, NT, D], BF16, tag="sinm")
        nc.sync.dma_start(
            out=cosf, in_=cos_p.ap().rearrange("(t p) d -> p t d", p=128))
        nc.sync.dma_start(
            out=sinm, in_=sin_p.ap().rearrange("(t p) d -> p t d", p=128))

        bc_tiles = {}
        for name in ("ln1_g", "ln1_b", "qn_g", "qn_b", "kn_g", "kn_b",
                     "ln2_g", "ln2_b", "b_out", "b2"):
            if name in vecs:
                t = const.tile([128, D], F32, tag=f"bc_{name}")
                nc.sync.dma_start(out=t, in_=bcast_ap(vecs[name], D))
                bc_tiles[name] = t
        if "b_qkv" in vecs:
            t = const.tile([128, D3], F32, tag="bc_bqkv")
            nc.sync.dma_start(out=t, in_=bcast_ap(vecs["b_qkv"], D3))
            bc_tiles["b_qkv"] = t
        for name in ("b1", "b3"):
            if name in vecs:
                t = const.tile([128, NH], F32, tag=f"col_{name}")
                ap = bass.AP(tensor=vecs[name].ap().tensor, offset=0,
                             ap=[[1, 128], [128, NH]])
                nc.sync.dma_start(out=t, in_=ap)
                bc_tiles[name] = t

        stat = stack.enter_context(tc.tile_pool(name="stat", bufs=4))
        xres = stack.enter_context(tc.tile_pool(name="xres", bufs=1))
        o1p = stack.enter_context(tc.tile_pool(name="o1p", bufs=1))

        x_N = [xres.tile([128, D], F32, tag=f"x{t}", name=f"x{t}")
               for t in range(NT)]
        out1_N = [o1p.tile([128, D], F32, tag=f"o1{t}", name=f"o1{t}")
                  for t in range(NT)]
        qkT_pool = stack.enter_context(tc.tile_pool(name="qkTp", bufs=1))
        q_T = qkT_pool.tile([128, ND, T], BF16, tag="qT", name="qT")
        k_T = qkT_pool.tile([128, ND, T], BF16, tag="kT", name="kT")

        def ln_stats(src_tile):
            """rstd [128,1], negm_r [128,1] for LN over D free elems."""
            st = stat.tile([128, 2, 6], F32, tag="lnst")
            nc.vector.bn_stats(out=st[:, 0, :], in_=src_tile[:, 0:D // 2])
            nc.vector.bn_stats(out=st[:, 1, :], in_=src_tile[:, D // 2:D])
            mv = stat.tile([128, 2], F32, tag="lnmv")
            nc.vector.bn_aggr(out=mv, in_=st)
            rstd = stat.tile([128, 1], F32, tag="lnrstd")
            nc.scalar.activation(out=rstd, in_=mv[:, 1:2], func=AF.Sqrt,
                                 bias=eps_t, scale=1.0, alpha=0.0)
            nc.vector.reciprocal(out=rstd, in_=rstd)
            negmr = stat.tile([128, 1], F32, tag="lnnm")
            nc.vector.tensor_mul(out=negmr, in0=mv[:, 0:1], in1=rstd)
            nc.scalar.mul(out=negmr, in_=negmr, mul=-1.0)
            return rstd, negmr

        # ---- Phase A: load x, LN1 -> h (bf16), DMA-transpose -> h_T ------
        hTp = tc.tile_pool(name="hTp", bufs=1)
        hT_pool = hTp.__enter__()
        h_T = hT_pool.tile([128, ND, T], BF16, tag="hT", name="hT")
        with tc.tile_pool(name="hpool", bufs=2) as hpool:
            for t in range(NT):
                nc.sync.dma_start(out=x_N[t],
                                  in_=x_p.ap()[t * 128:(t + 1) * 128, :])
                h_N = hpool.tile([128, D], BF16, tag="hN")
                rstd, negmr = ln_stats(x_N[t])
                nc.scalar.activation(out=h_N, in_=x_N[t], func=AF.Identity,
                                     scale=rstd, bias=negmr, alpha=0.0)
                if "ln1_g" in bc_tiles:
                    nc.vector.tensor_mul(out=h_N, in0=h_N,
                                         in1=bc_tiles["ln1_g"])
                    nc.vector.tensor_add(out=h_N, in0=h_N,
                                         in1=bc_tiles["ln1_b"])
                nc.scalar.dma_start_transpose(
                    out=h_T[:, :, t * 128:(t + 1) * 128], in_=h_N)

        # ---- Phase B/C: QKV projection, K/V first, gathers early ---------
        # wqkv_t storage order: [k0, k1, v0, v1, q0, q1]
        qknp_cm = tc.tile_pool(name="qknp", bufs=1)
        qknp = qknp_cm.__enter__()
        q_N = [qknp.tile([128, D], BF16, tag=f"qN{t}", name=f"qN{t}")
               for t in range(NT)]
        k_N = [qknp.tile([128, D], BF16, tag=f"kN{t}", name=f"kN{t}")
               for t in range(NT)]
        vpp = tc.tile_pool(name="vpp", bufs=1)
        vp_pool = vpp.__enter__()
        v_pad = vp_pool.tile([128, NT, H, HD + 1], FP8, tag="vpad", name="vpad")
        nc.vector.memset(v_pad[:, :, :, HD:HD + 1], 1.0)

        wq_cm = tc.tile_pool(name="wq", bufs=2)
        wq_pool = wq_cm.__enter__()
        mmps_cm = tc.tile_pool(name="mmps", bufs=2, space="PSUM")
        mmps = mmps_cm.__enter__()

        def qkv_chunk(ci):
            """ci: storage index into wqkv_t [k0,k1,v0,v1,q0,q1]."""
            w = wq_pool.tile([128, ND, 512], BF16, tag="wqt")
            nc.sync.dma_start(out=w, in_=wqkv_p.ap()[ci])
            ps = [mmps.tile([128, 512], F32, tag=f"qkvps{t}",
                            name=f"qkvps_{ci}_{t}") for t in range(NT)]
            for d in range(ND):
                for t in range(NT):
                    nc.tensor.matmul(
                        ps[t], h_T[:, d, t * 128:(t + 1) * 128], w[:, d, :],
                        start=(d == 0), stop=(d == ND - 1))
            # logical chunk: 0,1=q; 2,3=k; 4,5=v
            ch = [2, 3, 4, 5, 0, 1][ci]
            for t in range(NT):
                if ch < 2:
                    dst = q_N[t][:, (ch % 2) * 512:(ch % 2) * 512 + 512]
                elif ch < 4:
                    dst = k_N[t][:, (ch % 2) * 512:(ch % 2) * 512 + 512]
                else:
                    h0 = (ch - 4) * 8
                    dst = v_pad[:, t, h0:h0 + 8, 0:HD]
                    if "b_qkv" in bc_tiles:
                        nc.vector.tensor_add(
                            out=dst,
                            in0=bc_tiles["b_qkv"][:, ch * 512:(ch + 1) * 512]
                            .rearrange("p (h f) -> p h f", h=8),
                            in1=ps[t].rearrange("p (h f) -> p h f", h=8))
                    else:
                        nc.vector.tensor_copy(
                            out=dst,
                            in_=ps[t].rearrange("p (h f) -> p h f", h=8))
                    continue
                if "b_qkv" in bc_tiles:
                    nc.vector.tensor_add(
                        out=dst,
                        in0=bc_tiles["b_qkv"][:, ch * 512:(ch + 1) * 512],
                        in1=ps[t])
                else:
                    nc.vector.tensor_copy(out=dst, in_=ps[t])

        def qknorm_rope(src_N, dst_T, gname, t):
            """qk-norm + rope on [128, D] bf16, DMA-transpose into dst_T."""
            rstd, negmr = ln_stats(src_N)
            nrm = rope_pool.tile([128, D], BF16, tag="nrm")
            nc.scalar.activation(out=nrm, in_=src_N, func=AF.Identity,
                                 scale=rstd, bias=negmr, alpha=0.0)
            if f"{gname}_g" in bc_tiles:
                nc.vector.tensor_mul(out=nrm, in0=nrm,
                                     in1=bc_tiles[f"{gname}_g"])
                nc.vector.tensor_add(out=nrm, in0=nrm,
                                     in1=bc_tiles[f"{gname}_b"])
            nrm3 = nrm.rearrange("p (h f) -> p h f", h=H)
            sw = rope_pool.tile([128, H, HD], BF16, tag="sw")
            nc.vector.tensor_copy(out=sw[:, :, 0:32], in_=nrm3[:, :, 32:64])
            nc.vector.tensor_copy(out=sw[:, :, 32:64], in_=nrm3[:, :, 0:32])
            swf = sw.rearrange("p h f -> p (h f)")
            rp = rope_pool.tile([128, D], BF16, tag="rp")
            nc.vector.tensor_mul(out=rp, in0=nrm, in1=cosf[:, t, :])
            nc.vector.tensor_mul(out=swf, in0=swf, in1=sinm[:, t, :])
            nc.vector.tensor_add(out=rp, in0=rp, in1=swf)
            nc.scalar.dma_start_transpose(
                out=dst_T[:, :, t * 128:(t + 1) * 128], in_=rp)

        rope_cm = tc.tile_pool(name="ropep", bufs=2)
        rope_pool = rope_cm.__enter__()

        # K chunks -> k norm/rope -> bounce; V chunks -> bounce; one gather
        qkv_chunk(0)
        qkv_chunk(1)
        for t in range(NT):
            qknorm_rope(k_N[t], k_T, "kn", t)
        k_dst = bass.AP(tensor=kv_in.ap().tensor, offset=0,
                        ap=[[1024, 128], [128 * 1024, ND], [1, 1024]])
        nc.scalar.dma_start(out=k_dst, in_=k_T.bitcast(FP8))

        qkv_chunk(2)
        qkv_chunk(3)
        v_dst = bass.AP(tensor=kv_in.ap().tensor, offset=2 * KF,
                        ap=[[VW, 128], [128 * VW, NT], [1, VW]])
        nc.scalar.dma_start(out=v_dst,
                            in_=v_pad.rearrange("p t h f -> p t (h f)"))
        nc.gpsimd.collective_compute(
            "AllGather", mybir.AluOpType.bypass, replica_groups=groups,
            ins=[kv_in.ap().opt()], outs=[kv_all.ap().opt()])

        # Q chunks + q norm/rope (overlaps the gather)
        qkv_chunk(4)
        qkv_chunk(5)
        for t in range(NT):
            qknorm_rope(q_N[t], q_T, "qn", t)

        rope_cm.__exit__(None, None, None)
        mmps_cm.__exit__(None, None, None)
        wq_cm.__exit__(None, None, None)
        vpp.__exit__(None, None, None)
        qknp_cm.__exit__(None, None, None)
        hTp.__exit__(None, None, None)

        # ---- Phase E: attention ------------------------------------------
        h2Tp = stack.enter_context(tc.tile_pool(name="h2Tp", bufs=1))
        h2_T = h2Tp.tile([128, ND, T], BF16, tag="h2T", name="h2T")
        stackp_cm = tc.tile_pool(name="stackp", bufs=1)
        stackp = stackp_cm.__enter__()
        stacked = stackp.tile([128, ND, T], BF16, tag="stk", name="stk")

        kvrem_cm = tc.tile_pool(name="kvrem", bufs=1)
        kvrem = kvrem_cm.__enter__()
        k_r = []
        v_r = []
        for rc in range(GROUP):
            kt_ = kvrem.tile([128, ND, T], BF16, tag=f"kr{rc}", name=f"kr{rc}")
            src = bass.AP(tensor=kv_all.ap().tensor, offset=rc * KVB,
                          ap=[[1024, 128], [128 * 1024, ND], [1, 1024]])
            nc.scalar.dma_start(out=kt_.bitcast(FP8), in_=src)
            k_r.append(kt_)
            vt_ = kvrem.tile([128, NT, H, HD + 1], FP8, tag=f"vr{rc}",
                             name=f"vr{rc}")
            src = bass.AP(tensor=kv_all.ap().tensor,
                          offset=rc * KVB + 2 * KF,
                          ap=[[VW, 128], [128 * VW, NT], [1, VW]])
            nc.scalar.dma_start(out=vt_.rearrange("p t h f -> p t (h f)"),
                                in_=src)
            v_r.append(vt_)

        with (
            tc.tile_pool(name="scps", bufs=2, space="PSUM") as scps,
            tc.tile_pool(name="pvps", bufs=1, space="PSUM") as pvps,
            tc.tile_pool(name="bcps", bufs=2, space="PSUM") as bcps,
            tc.tile_pool(name="scsb", bufs=3) as scsb,
            tc.tile_pool(name="prb", bufs=3) as prb,
            tc.tile_pool(name="accp", bufs=2) as accp,
            tc.tile_pool(name="tbp", bufs=2) as tbp,
        ):
            for d in range(ND):
                pvA = pvps.tile([65, T], F32, tag="pvA", name=f"pvA{d}")
                pvB = pvps.tile([65, T], F32, tag="pvB", name=f"pvB{d}")
                for half, pv in (("A", pvA), ("B", pvB)):
                    hoff = 0 if half == "A" else 64
                    hidx = 2 * d if half == "A" else 2 * d + 1
                    # software pipeline: scores/exp for pair p, PV for p-1
                    prs = [None] * NPAIR

                    def scores(p):
                        rc, pl = divmod(p, NT // 2)
                        ps = scps.tile([128, 2, T], F32, tag="scp",
                                       name=f"sc_{d}_{half}_{p}")
                        scs = scsb.tile([128, 2, T], BF16, tag="scs",
                                        name=f"scs_{d}_{half}_{p}")
                        pr = prb.tile([128, 2, T], FP8, tag="prp",
                                      name=f"pr_{d}_{half}_{p}")
                        for j in range(2):
                            kt = pl * 2 + j
                            sl = slice(kt * 128, (kt + 1) * 128)
                            nc.tensor.matmul(
                                ps[:, j, :],
                                k_r[rc][hoff:hoff + 64, d, sl],
                                q_T[hoff:hoff + 64, d, :],
                                start=True, stop=True,
                                tile_position=(hoff, 0))
                            # ACT reads PSUM ~5x slower than SBUF: bounce
                            # scores through SBUF on the vector engine.
                            nc.vector.tensor_copy(out=scs[:, j, :],
                                                  in_=ps[:, j, :])
                            nc.scalar.activation(
                                out=pr[:, j, :], in_=scs[:, j, :],
                                func=AF.Exp, scale=1.0 / np.sqrt(HD),
                                alpha=0.0)
                        prs[p] = pr

                    def pv_acc(p):
                        rc, pl = divmod(p, NT // 2)
                        vsl = v_r[rc][:, pl * 2:pl * 2 + 2, hidx, :]
                        nc.tensor.matmul(
                            pv, vsl, prs[p],
                            start=(p == 0), stop=(p == NPAIR - 1),
                            perf_mode=DR)

                    scores(0)
                    for p in range(NPAIR):
                        if p + 1 < NPAIR:
                            scores(p + 1)
                        pv_acc(p)

                # denominators: broadcast row 64, reciprocal, multiply
                for half, pv in (("A", pvA), ("B", pvB)):
                    acc = accp.tile([65, T], F32, tag="acc")
                    nc.vector.tensor_copy(out=acc.bitcast(F32R), in_=pv)
                    bc = bcps.tile([64, T], F32, tag="bc")
                    nc.tensor.matmul(bc, sel.bitcast(F32R), acc.bitcast(F32R),
                                     start=True, stop=True)
                    rec = tbp.tile([64, T], F32, tag="rec")
                    nc.vector.reciprocal(out=rec, in_=bc)
                    if half == "A":
                        nc.vector.tensor_mul(out=stacked[0:64, d, :],
                                             in0=acc[0:64, :], in1=rec)
                    else:
                        tmpB = tbp.tile([64, T], BF16, tag="tmpB")
                        nc.vector.tensor_mul(out=tmpB, in0=acc[0:64, :],
                                             in1=rec)
                        nc.sync.dma_start(out=stacked[64:128, d, :], in_=tmpB)

        kvrem_cm.__exit__(None, None, None)

        # ---- Phase F: out projection + residual + LN2, pipelined per t ---
        with (
            tc.tile_pool(name="wo", bufs=1) as wo,
            tc.tile_pool(name="ops", bufs=2, space="PSUM") as ops,
            tc.tile_pool(name="h2pool", bufs=2) as h2pool,
        ):
            wot = []
            for ch in range(2):
                w = wo.tile([128, ND, 512], BF16, tag=f"wot{ch}")
                nc.sync.dma_start(out=w, in_=wout_p.ap()[ch])
                wot.append(w)
            for t in range(NT):
                ps = [ops.tile([128, 512], F32, tag=f"ops{ch}",
                               name=f"ops_{ch}_{t}") for ch in range(2)]
                for d in range(ND):
                    for ch in range(2):
                        nc.tensor.matmul(
                            ps[ch], stacked[:, d, t * 128:(t + 1) * 128],
                            wot[ch][:, d, :],
                            start=(d == 0), stop=(d == ND - 1))
                for ch in range(2):
                    sl = slice(ch * 512, (ch + 1) * 512)
                    nc.vector.tensor_add(out=out1_N[t][:, sl],
                                         in0=x_N[t][:, sl], in1=ps[ch])
                    if "b_out" in bc_tiles:
                        nc.vector.tensor_add(out=out1_N[t][:, sl],
                                             in0=out1_N[t][:, sl],
                                             in1=bc_tiles["b_out"][:, sl])
                h2_N = h2pool.tile([128, D], BF16, tag="h2N")
                rstd, negmr = ln_stats(out1_N[t])
                nc.scalar.activation(out=h2_N, in_=out1_N[t], func=AF.Identity,
                                     scale=rstd, bias=negmr, alpha=0.0)
                if "ln2_g" in bc_tiles:
                    nc.vector.tensor_mul(out=h2_N, in0=h2_N,
                                         in1=bc_tiles["ln2_g"])
                    nc.vector.tensor_add(out=h2_N, in0=h2_N,
                                         in1=bc_tiles["ln2_b"])
                nc.scalar.dma_start_transpose(
                    out=h2_T[:, :, t * 128:(t + 1) * 128], in_=h2_N)

        stackp_cm.__exit__(None, None, None)

        # ---- Phase H: FFN1/FFN3 -> prod_T --------------------------------
        prp = stack.enter_context(tc.tile_pool(name="prp", bufs=1))
        prod_T = [prp.tile([128, T], BF16, tag=f"pr{h}", name=f"pr{h}")
                  for h in range(NH)]
        with (
            tc.tile_pool(name="wf", bufs=3) as wf,
            tc.tile_pool(name="ffps", bufs=2, space="PSUM") as ffps,
            tc.tile_pool(name="s1p", bufs=2) as s1p,
        ):
            for ht in range(NH):
                w1sb = wf.tile([128, ND, 128], BF16, tag="w1sb")
                w3sb = wf.tile([128, ND, 128], BF16, tag="w3sb")
                nc.sync.dma_start(out=w1sb, in_=w1_p.ap()[ht])
                nc.sync.dma_start(out=w3sb, in_=w3_p.ap()[ht])
                ps1 = ffps.tile([128, T], F32, tag="ps1")
                ps3 = ffps.tile([128, T], F32, tag="ps3")
                for d in range(ND):
                    nc.tensor.matmul(ps1, w1sb[:, d, :], h2_T[:, d, :],
                                     start=(d == 0), stop=(d == ND - 1))
                for d in range(ND):
                    nc.tensor.matmul(ps3, w3sb[:, d, :], h2_T[:, d, :],
                                     start=(d == 0), stop=(d == ND - 1))
                s1 = s1p.tile([128, T], BF16, tag="s1")
                b1arg = bc_tiles["b1"][:, ht:ht + 1] if "b1" in bc_tiles else 0.0
                nc.scalar.activation(out=s1, in_=ps1, func=AF.Silu,
                                     bias=b1arg, scale=1.0, alpha=0.0)
                t3 = s1p.tile([128, T], BF16, tag="t3")
                if "b3" in bc_tiles:
                    nc.vector.tensor_scalar_add(
                        out=t3, in0=ps3, scalar1=bc_tiles["b3"][:, ht:ht + 1])
                else:
                    nc.vector.tensor_copy(out=t3, in_=ps3)
                nc.vector.tensor_mul(out=prod_T[ht], in0=s1, in1=t3)

        # ---- Phase I: FFN2 + residual + store ----------------------------
        with (
            tc.tile_pool(name="w2p", bufs=2) as w2p,
            tc.tile_pool(name="f2ps", bufs=1, space="PSUM") as f2ps,
        ):
            ps = [[f2ps.tile([128, 512], F32, tag=f"f2_{t}_{ch}",
                             name=f"f2_{t}_{ch}") for ch in range(2)]
                  for t in range(NT)]
            for hg in range(NH // 8):
                w2g = w2p.tile([128, 8, D], BF16, tag="w2g")
                nc.sync.dma_start(out=w2g, in_=w2_p.ap()[hg])
                for hi in range(8):
                    ht = hg * 8 + hi
                    for t in range(NT):
                        for ch in range(2):
                            nc.tensor.matmul(
                                ps[t][ch],
                                prod_T[ht][:, t * 128:(t + 1) * 128],
                                w2g[:, hi, ch * 512:(ch + 1) * 512],
                                start=(ht == 0), stop=(ht == NH - 1))
            for t in range(NT):
                for ch in range(2):
                    sl = slice(ch * 512, (ch + 1) * 512)
                    nc.vector.tensor_add(out=out1_N[t][:, sl],
                                         in0=out1_N[t][:, sl], in1=ps[t][ch])
                    if "b2" in bc_tiles:
                        nc.vector.tensor_add(out=out1_N[t][:, sl],
                                             in0=out1_N[t][:, sl],
                                             in1=bc_tiles["b2"][:, sl])
                nc.sync.dma_start(out=out_p.ap()[t * 128:(t + 1) * 128, :],
                                  in_=out1_N[t])

    _split_all_waits(nc)
    return nc


# ---------------------------------------------------------------------------
# Host wrapper
# ---------------------------------------------------------------------------

_CACHE = {}
_PREP_CACHE = {}


def _prep_inputs(x, rope_cos, rope_sin, w_qkv, b_qkv, w_out, b_out,
                 qn_g, qn_b, kn_g, kn_b, ln1_g, ln1_b, ln2_g, ln2_b,
                 w1, b1, w2, b2, w3, b3):
    B, S, D = x.shape
    H, HD, FFN = 16, 64, 4096
    T = B * S // N_CORES
    ND, NH = D // 128, FFN // 128

    wkey = (id(w_qkv), id(w_out), id(w1), id(w2), id(w3),
            id(rope_cos), id(rope_sin))
    if wkey in _PREP_CACHE:
        shared, flags = _PREP_CACHE[wkey]
    else:
        flags = set()
        if not (np.all(ln1_g == 1) and np.all(ln1_b == 0)):
            flags.add("ln1_gb")
        if not (np.all(qn_g == 1) and np.all(qn_b == 0)):
            flags.add("qn_gb")
        if not (np.all(kn_g == 1) and np.all(kn_b == 0)):
            flags.add("kn_gb")
        if not (np.all(ln2_g == 1) and np.all(ln2_b == 0)):
            flags.add("ln2_gb")
        if np.any(b_qkv != 0):
            flags.add("bqkv")
        if np.any(b_out != 0):
            flags.add("bout")
        if np.any(b1 != 0):
            flags.add("b1")
        if np.any(b2 != 0):
            flags.add("b2")
        if np.any(b3 != 0):
            flags.add("b3")
        flags = frozenset(flags)

        bf = ml_dtypes.bfloat16
        # rope tables [S, D]: cos tiled over heads; sin with sign folded
        cosfull = np.tile(rope_cos, (1, H)).astype(bf)
        sinmod_half = np.concatenate(
            [-rope_sin[:, :HD // 2], rope_sin[:, HD // 2:]], axis=1)
        sinmod = np.tile(sinmod_half, (1, H)).astype(bf)

        wqkvT = np.ascontiguousarray(w_qkv.T)           # [D, 3D]
        # wqkv_t [6, 128, ND, 512]: storage order [k0,k1,v0,v1,q0,q1];
        # [ci, p, d, c] = wqkvT[d*128+p, ch*512+c]
        ch_order = [2, 3, 4, 5, 0, 1]
        wq4 = wqkvT.reshape(ND, 128, 6, 512)            # [d, p, ch, c]
        wqkv_t = np.ascontiguousarray(
            wq4.transpose(2, 1, 0, 3)[ch_order]).astype(bf)
        woutT = np.ascontiguousarray(w_out.T)           # [D, D]
        wo4 = woutT.reshape(ND, 128, 2, 512)
        wout_t = np.ascontiguousarray(wo4.transpose(2, 1, 0, 3)).astype(bf)
        # w1_t [NH, 128, ND, 128]: [ht, p, d, c] = w1[ht*128+c, d*128+p]
        w1r = w1.reshape(NH, 128, ND, 128)              # [ht, c, d, p]
        w1_t = np.ascontiguousarray(w1r.transpose(0, 3, 2, 1)).astype(bf)
        w3r = w3.reshape(NH, 128, ND, 128)
        w3_t = np.ascontiguousarray(w3r.transpose(0, 3, 2, 1)).astype(bf)
        # w2_t [NH//8, 128, 8, D]: [hg, p, hi, c] = w2[c, (hg*8+hi)*128+p]
        w2r = w2.reshape(D, NH // 8, 8, 128)            # [c, hg, hi, p]
        w2_t = np.ascontiguousarray(w2r.transpose(1, 3, 2, 0)).astype(bf)

        shared = {
            "wqkv_t": wqkv_t, "wout_t": wout_t,
            "w1_t": w1_t, "w3_t": w3_t, "w2_t": w2_t,
            "cosfull": cosfull, "sinmod": sinmod,
            # keep refs so ids stay unique
            "_refs": (w_qkv, w_out, w1, w2, w3, rope_cos, rope_sin),
        }
        opt = {"ln1_gb": [("ln1_g", ln1_g), ("ln1_b", ln1_b)],
               "qn_gb": [("qn_g", qn_g), ("qn_b", qn_b)],
               "kn_gb": [("kn_g", kn_g), ("kn_b", kn_b)],
               "ln2_gb": [("ln2_g", ln2_g), ("ln2_b", ln2_b)],
               "bqkv": [("b_qkv", b_qkv)], "bout": [("b_out", b_out)],
               "b1": [("b1", b1)], "b2": [("b2", b2)], "b3": [("b3", b3)]}
        for fl, items in opt.items():
            if fl in flags:
                for name, arr in items:
                    shared[name] = np.ascontiguousarray(arr).astype(np.float32)
        _PREP_CACHE[wkey] = (shared, flags)

    xf = np.ascontiguousarray(x.reshape(B * S, D)).astype(np.float32)
    in_maps = []
    for c in range(N_CORES):
        t0 = c * T
        m = {k: v for k, v in shared.items() if k != "_refs"}
        m["x"] = xf[t0:t0 + T]
        m["cosfull"] = shared["cosfull"][t0 % S:t0 % S + T]
        m["sinmod"] = shared["sinmod"][t0 % S:t0 % S + T]
        in_maps.append(m)
    return in_maps, flags, T, D


def kernel(**inputs):
    x = inputs["x"]
    B, S, D = x.shape
    in_maps, flags, T, _ = _prep_inputs(**inputs)

    key = (T, D, flags)
    if key not in _CACHE:
        _CACHE[key] = build_nc(T=T, D=D, flags=flags)
    nc = _CACHE[key]

    res = run_bass_kernel_spmd(nc, in_maps, core_ids=list(range(N_CORES)))
    out = np.empty((B * S, D), np.float32)
    for c in range(N_CORES):
        out[c * T:(c + 1) * T] = res.results[c]["out"]
    return out.reshape(B, S, D)


# revision 32
# speedup vs baseline: 1.0560x; 1.0560x over previous
"""Trainium2 Bass kernel for the pre-norm attention + SwiGLU FFN layer (v2).

Sharding: tokens (batch*seq flattened) split across 8 cores - 512 tokens
each; cores 0-3 hold batch 0, cores 4-7 batch 1. All per-token work (LNs,
projections, rope, FFN) is local with replicated weights; attention gathers
rope'd K (bf16) and ones-padded V (fp8e4m3) across each 4-core batch group
with two AllGathers (K first so scores can start while V is in flight),
then each core attends its 512 queries over the full 2048 context.

v2 vs v1:
 - all matmuls in bf16 (same PE rate as f32r, half the bytes); PV matmul in
   fp8 DoubleRow (2x PE rate; exp(scores) stays in [e^-5, e^4.6], inside
   e4m3 range, ones column exact).
 - transposes done by the DMA XBAR (dma_start_transpose, 2-byte dtype)
   instead of PE transposes + vector copy-backs.
 - weights are host-side pre-tiled so every weight DMA is contiguous per
   partition (2-16 KB lines instead of the 512 B packet storm).
 - denominators: ones-row PV output broadcast via one sel-matmul, one
   vector reciprocal on 64 partitions, one multiply (vs 32 single-partition
   reciprocals).
 - K/V projection and rope run before Q's so the collectives trigger early;
   Q-side prep overlaps the gathers.
 - attention inner loop is software-pipelined (scores for pair p+1 issued
   before the PV of pair p) to keep the PE p-state up.
"""

import numpy as np
import ml_dtypes

import bass_rust
import concourse.bass as bass
import concourse.mybir as mybir
import concourse.tile as tile
from concourse.bass_utils import run_bass_kernel_spmd
from concourse.vector_clock import ScopedClock

F32 = mybir.dt.float32
F32R = mybir.dt.float32r
BF16 = mybir.dt.bfloat16
FP8 = mybir.dt.float8e4
AF = mybir.ActivationFunctionType
DR = mybir.MatmulPerfMode.DoubleRow

N_CORES = 8
GROUP = 4
EPS = 1e-6

# ---------------------------------------------------------------------------
# Workaround for this walrus build's 1-wait-per-instruction encoding limit.
# ---------------------------------------------------------------------------
_MAX_WAITS = 1
_carrier_id = [0]


def _patched_drain_and_barrier(self, tick_clock, wait_clock):
    nc = self.nc
    drain_inst = nc.sync.drain()
    wait_clock.add_sem_waits(
        drain_inst.ins, ScopedClock({None: tick_clock.global_clock})
    )
    si = drain_inst.ins.sync_info
    waits = list(si.on_wait)
    if len(waits) > _MAX_WAITS:
        drain_inst.ins.sync_info = bass_rust.SyncInfo(
            on_wait=waits[:_MAX_WAITS], on_update=list(si.on_update)
        )
        rest = waits[_MAX_WAITS:]
        while rest:
            chunk, rest = rest[:_MAX_WAITS], rest[_MAX_WAITS:]
            extra = nc.sync.drain()
            extra.ins.sync_info = bass_rust.SyncInfo(on_wait=chunk, on_update=[])

    nc.all_engine_barrier()
    assert self.sems is not None
    popped = nc._tile_sem_poison_stack.pop()
    assert popped is self._sem_poison
    nc.clear_and_free_semaphores(list(self.sems.allocated().values()))
    nc.all_engine_barrier()


tile.TileContext._drain_and_barrier = _patched_drain_and_barrier


def _split_all_waits(nc, max_waits=_MAX_WAITS):
    for fn in nc.m.functions:
        for bb in fn.blocks:
            insts = list(bb.instructions)
            out = []
            changed = False
            for inst in insts:
                si = getattr(inst, "sync_info", None)
                if si is not None and si.on_wait and len(si.on_wait) > max_waits:
                    waits = list(si.on_wait)
                    updates = list(si.on_update)
                    extra, keep = waits[:-max_waits], waits[-max_waits:]
                    while extra:
                        chunk, extra = extra[:max_waits], extra[max_waits:]
                        _carrier_id[0] += 1
                        nop = mybir.InstNoOp(name=f"I-waitcar-{_carrier_id[0]}")
                        nop.engine = inst.engine
                        nop.sync_info = bass_rust.SyncInfo(on_wait=chunk, on_update=[])
                        nc.register_instruction(nop)
                        out.append(nop)
                    inst.sync_info = bass_rust.SyncInfo(on_wait=keep, on_update=updates)
                    changed = True
                out.append(inst)
            if changed:
                bb.instructions = out


# ---------------------------------------------------------------------------
# Graph builder (one SPMD program for all 8 cores)
# ---------------------------------------------------------------------------

def build_nc(T=512, D=1024, H=16, HD=64, FFN=4096, flags=frozenset()):
    NT = T // 128            # token tiles per core (4)
    ND = D // 128            # model-dim tiles (8)
    NH = FFN // 128          # ffn hidden tiles (32)
    D3 = 3 * D
    CTX = GROUP * T          # context tokens (2048)
    NKT = CTX // 128         # context k-token tiles (16)
    NPAIR = NKT // 2         # kt pairs for DoubleRow PV (8)
    VW = H * (HD + 1)        # padded v width per token (1040)
    KF = D * T               # k floats per rank
    VF = T * VW              # v elements per rank

    nc = bass.Bass(trn_type="TRN2", num_devices=N_CORES)

    x_p = nc.declare_dram_parameter("x", [T, D], F32, isOutput=False)
    cos_p = nc.declare_dram_parameter("cosfull", [T, D], BF16, isOutput=False)
    sin_p = nc.declare_dram_parameter("sinmod", [T, D], BF16, isOutput=False)
    # pre-tiled weights (see _prep_inputs for layouts)
    wqkv_p = nc.declare_dram_parameter("wqkv_t", [6, 128, ND, 512], BF16,
                                       isOutput=False)
    wout_p = nc.declare_dram_parameter("wout_t", [2, 128, ND, 512], BF16,
                                       isOutput=False)
    w1_p = nc.declare_dram_parameter("w1_t", [NH, 128, ND, 128], BF16,
                                     isOutput=False)
    w3_p = nc.declare_dram_parameter("w3_t", [NH, 128, ND, 128], BF16,
                                     isOutput=False)
    w2_p = nc.declare_dram_parameter("w2_t", [NH // 8, 128, 8, D], BF16,
                                     isOutput=False)
    vecs = {}
    for name, size in [("ln1_g", D), ("ln1_b", D), ("qn_g", D), ("qn_b", D),
                       ("kn_g", D), ("kn_b", D), ("ln2_g", D), ("ln2_b", D),
                       ("b_qkv", D3), ("b_out", D), ("b1", FFN), ("b3", FFN),
                       ("b2", D)]:
        flag = {"ln1_g": "ln1_gb", "ln1_b": "ln1_gb", "qn_g": "qn_gb",
                "qn_b": "qn_gb", "kn_g": "kn_gb", "kn_b": "kn_gb",
                "ln2_g": "ln2_gb", "ln2_b": "ln2_gb", "b_qkv": "bqkv",
                "b_out": "bout", "b1": "b1", "b3": "b3", "b2": "b2"}[name]
        if flag in flags:
            vecs[name] = nc.declare_dram_parameter(name, [size], F32,
                                                   isOutput=False)
    out_p = nc.declare_dram_parameter("out", [T, D], F32, isOutput=True)

    KVB = (2 * KF + VF) // 4  # f32 elems per rank (k bf16 + v fp8, packed)
    kv_in = nc.dram_tensor("kv_in", [KVB], F32)
    kv_all = nc.dram_tensor("kv_all", [GROUP * KVB], F32)

    groups = [list(range(g * GROUP, (g + 1) * GROUP))
              for g in range(N_CORES // GROUP)]

    def bcast_ap(param, width):
        return bass.AP(tensor=param.ap().tensor, offset=0,
                       ap=[[0, 128], [1, width]])

    from contextlib import ExitStack
    with tile.TileContext(nc) as tc, ExitStack() as stack:
        const = stack.enter_context(tc.tile_pool(name="const", bufs=1))
        sel = const.tile([65, 64], F32, tag="sel")
        nc.vector.memset(sel, 0.0)
        nc.vector.memset(sel[64:65, :], 1.0)
        eps_t = const.tile([128, 1], F32, tag="eps")
        nc.vector.memset(eps_t, EPS)
        cosf = const.tile([128, NT, D], BF16, tag="cosf")
        sinm = const.tile([128, NT, D], BF16, tag="sinm")
        nc.sync.dma_start(
            out=cosf, in_=cos_p.ap().rearrange("(t p) d -> p t d", p=128))
        nc.sync.dma_start(
            out=sinm, in_=sin_p.ap().rearrange("(t p) d -> p t d", p=128))

        bc_tiles = {}
        for name in ("ln1_g", "ln1_b", "qn_g", "qn_b", "kn_g", "kn_b",
                     "ln2_g", "ln2_b", "b_out", "b2"):
            if name in vecs:
                t = const.tile([128, D], F32, tag=f"bc_{name}")
                nc.sync.dma_start(out=t, in_=bcast_ap(vecs[name], D))
                bc_tiles[name] = t
        if "b_qkv" in vecs:
            t = const.tile([128, D3], F32, tag="bc_bqkv")
            nc.sync.dma_start(out=t, in_=bcast_ap(vecs["b_qkv"], D3))
            bc_tiles["b_qkv"] = t
        for name in ("b1", "b3"):
            if name in vecs:
                t = const.tile([128, NH], F32, tag=f"col_{name}")
                ap = bass.AP(tensor=vecs[name].ap().tensor, offset=0,
                             ap=[[1, 128], [128, NH]])
                nc.sync.dma_start(out=t, in_=ap)
                bc_tiles[name] = t

        stat = stack.enter_context(tc.tile_pool(name="stat", bufs=4))
        xres = stack.enter_context(tc.tile_pool(name="xres", bufs=1))
        o1p = stack.enter_context(tc.tile_pool(name="o1p", bufs=1))

        x_N = [xres.tile([128, D], F32, tag=f"x{t}", name=f"x{t}")
               for t in range(NT)]
        out1_N = [o1p.tile([128, D], F32, tag=f"o1{t}", name=f"o1{t}")
                  for t in range(NT)]
        qkT_pool = stack.enter_context(tc.tile_pool(name="qkTp", bufs=1))
        q_T = qkT_pool.tile([128, ND, T], BF16, tag="qT", name="qT")
        k_T = qkT_pool.tile([128, ND, T], BF16, tag="kT", name="kT")

        def ln_stats(src_tile):
            """rstd [128,1], negm_r [128,1] for LN over D free elems."""
            st = stat.tile([128, 2, 6], F32, tag="lnst")
            nc.vector.bn_stats(out=st[:, 0, :], in_=src_tile[:, 0:D // 2])
            nc.vector.bn_stats(out=st[:, 1, :], in_=src_tile[:, D // 2:D])
            mv = stat.tile([128, 2], F32, tag="lnmv")
            nc.vector.bn_aggr(out=mv, in_=st)
            rstd = stat.tile([128, 1], F32, tag="lnrstd")
            nc.scalar.activation(out=rstd, in_=mv[:, 1:2], func=AF.Sqrt,
                                 bias=eps_t, scale=1.0, alpha=0.0)
            nc.vector.reciprocal(out=rstd, in_=rstd)
            negmr = stat.tile([128, 1], F32, tag="lnnm")
            nc.vector.tensor_mul(out=negmr, in0=mv[:, 0:1], in1=rstd)
            nc.scalar.mul(out=negmr, in_=negmr, mul=-1.0)
            return rstd, negmr

        # ---- Phase A: load x, LN1 -> h (bf16), DMA-transpose -> h_T ------
        hTp = tc.tile_pool(name="hTp", bufs=1)
        hT_pool = hTp.__enter__()
        h_T = hT_pool.tile([128, ND, T], BF16, tag="hT", name="hT")
        with tc.tile_pool(name="hpool", bufs=2) as hpool:
            for t in range(NT):
                nc.sync.dma_start(out=x_N[t],
                                  in_=x_p.ap()[t * 128:(t + 1) * 128, :])
                h_N = hpool.tile([128, D], BF16, tag="hN")
                rstd, negmr = ln_stats(x_N[t])
                nc.scalar.activation(out=h_N, in_=x_N[t], func=AF.Identity,
                                     scale=rstd, bias=negmr, alpha=0.0)
                if "ln1_g" in bc_tiles:
                    nc.vector.tensor_mul(out=h_N, in0=h_N,
                                         in1=bc_tiles["ln1_g"])
                    nc.vector.tensor_add(out=h_N, in0=h_N,
                                         in1=bc_tiles["ln1_b"])
                nc.scalar.dma_start_transpose(
                    out=h_T[:, :, t * 128:(t + 1) * 128], in_=h_N)

        # ---- Phase B/C: QKV projection, K/V first, gathers early ---------
        # wqkv_t storage order: [k0, k1, v0, v1, q0, q1]
        qknp_cm = tc.tile_pool(name="qknp", bufs=1)
        qknp = qknp_cm.__enter__()
        q_N = [qknp.tile([128, D], BF16, tag=f"qN{t}", name=f"qN{t}")
               for t in range(NT)]
        k_N = [qknp.tile([128, D], BF16, tag=f"kN{t}", name=f"kN{t}")
               for t in range(NT)]
        vpp = tc.tile_pool(name="vpp", bufs=1)
        vp_pool = vpp.__enter__()
        v_flat = vp_pool.tile([128, NT, VW], FP8, tag="vpad", name="vpad")
        v_pad = v_flat.rearrange("p t (h f) -> p t h f", h=H)
        nc.vector.memset(v_pad[:, :, :, HD:HD + 1], 1.0)

        wq_cm = tc.tile_pool(name="wq", bufs=2)
        wq_pool = wq_cm.__enter__()
        mmps_cm = tc.tile_pool(name="mmps", bufs=2, space="PSUM")
        mmps = mmps_cm.__enter__()

        def qkv_chunk(ci):
            """ci: storage index into wqkv_t [k0,k1,v0,v1,q0,q1]."""
            w = wq_pool.tile([128, ND, 512], BF16, tag="wqt")
            nc.sync.dma_start(out=w, in_=wqkv_p.ap()[ci])
            ps = [mmps.tile([128, 512], F32, tag=f"qkvps{t}",
                            name=f"qkvps_{ci}_{t}") for t in range(NT)]
            for d in range(ND):
                for t in range(NT):
                    nc.tensor.matmul(
                        ps[t], h_T[:, d, t * 128:(t + 1) * 128], w[:, d, :],
                        start=(d == 0), stop=(d == ND - 1))
            # logical chunk: 0,1=q; 2,3=k; 4,5=v
            ch = [2, 3, 4, 5, 0, 1][ci]
            for t in range(NT):
                if ch < 2:
                    dst = q_N[t][:, (ch % 2) * 512:(ch % 2) * 512 + 512]
                elif ch < 4:
                    dst = k_N[t][:, (ch % 2) * 512:(ch % 2) * 512 + 512]
                else:
                    h0 = (ch - 4) * 8
                    dst = v_pad[:, t, h0:h0 + 8, 0:HD]
                    if "b_qkv" in bc_tiles:
                        nc.vector.tensor_add(
                            out=dst,
                            in0=bc_tiles["b_qkv"][:, ch * 512:(ch + 1) * 512]
                            .rearrange("p (h f) -> p h f", h=8),
                            in1=ps[t].rearrange("p (h f) -> p h f", h=8))
                    else:
                        nc.vector.tensor_copy(
                            out=dst,
                            in_=ps[t].rearrange("p (h f) -> p h f", h=8))
                    continue
                if "b_qkv" in bc_tiles:
                    nc.vector.tensor_add(
                        out=dst,
                        in0=bc_tiles["b_qkv"][:, ch * 512:(ch + 1) * 512],
                        in1=ps[t])
                else:
                    nc.vector.tensor_copy(out=dst, in_=ps[t])

        def qknorm_rope(src_N, dst_T, gname, t):
            """qk-norm + rope on [128, D] bf16, DMA-transpose into dst_T."""
            rstd, negmr = ln_stats(src_N)
            nrm = rope_pool.tile([128, D], BF16, tag="nrm")
            nc.scalar.activation(out=nrm, in_=src_N, func=AF.Identity,
                                 scale=rstd, bias=negmr, alpha=0.0)
            if f"{gname}_g" in bc_tiles:
                nc.vector.tensor_mul(out=nrm, in0=nrm,
                                     in1=bc_tiles[f"{gname}_g"])
                nc.vector.tensor_add(out=nrm, in0=nrm,
                                     in1=bc_tiles[f"{gname}_b"])
            nrm3 = nrm.rearrange("p (h f) -> p h f", h=H)
            sw = rope_pool.tile([128, H, HD], BF16, tag="sw")
            nc.vector.tensor_copy(out=sw[:, :, 0:32], in_=nrm3[:, :, 32:64])
            nc.vector.tensor_copy(out=sw[:, :, 32:64], in_=nrm3[:, :, 0:32])
            swf = sw.rearrange("p h f -> p (h f)")
            rp = rope_pool.tile([128, D], BF16, tag="rp")
            nc.vector.tensor_mul(out=rp, in0=nrm, in1=cosf[:, t, :])
            nc.vector.tensor_mul(out=swf, in0=swf, in1=sinm[:, t, :])
            nc.vector.tensor_add(out=rp, in0=rp, in1=swf)
            nc.scalar.dma_start_transpose(
                out=dst_T[:, :, t * 128:(t + 1) * 128], in_=rp)

        rope_cm = tc.tile_pool(name="ropep", bufs=2)
        rope_pool = rope_cm.__enter__()

        # K chunks -> k norm/rope -> bounce; V chunks -> bounce; one gather
        qkv_chunk(0)
        qkv_chunk(1)
        for t in range(NT):
            qknorm_rope(k_N[t], k_T, "kn", t)
        k_dst = bass.AP(tensor=kv_in.ap().tensor, offset=0,
                        ap=[[256, 128], [128 * 256, ND], [1, 256]])
        nc.scalar.dma_start(out=k_dst, in_=k_T.bitcast(F32))

        qkv_chunk(2)
        qkv_chunk(3)
        v_dst = bass.AP(tensor=kv_in.ap().tensor, offset=KF // 2,
                        ap=[[VW // 4, 128], [128 * VW // 4, NT], [1, VW // 4]])
        nc.scalar.dma_start(out=v_dst, in_=v_flat.bitcast(F32))
        nc.gpsimd.collective_compute(
            "AllGather", mybir.AluOpType.bypass, replica_groups=groups,
            ins=[kv_in.ap().opt()], outs=[kv_all.ap().opt()])

        # Q chunks + q norm/rope (overlaps the gather)
        qkv_chunk(4)
        qkv_chunk(5)
        for t in range(NT):
            qknorm_rope(q_N[t], q_T, "qn", t)

        rope_cm.__exit__(None, None, None)
        mmps_cm.__exit__(None, None, None)
        wq_cm.__exit__(None, None, None)
        vpp.__exit__(None, None, None)
        qknp_cm.__exit__(None, None, None)
        hTp.__exit__(None, None, None)

        # ---- Phase E: attention ------------------------------------------
        h2Tp = stack.enter_context(tc.tile_pool(name="h2Tp", bufs=1))
        h2_T = h2Tp.tile([128, ND, T], BF16, tag="h2T", name="h2T")
        stackp_cm = tc.tile_pool(name="stackp", bufs=1)
        stackp = stackp_cm.__enter__()
        stacked = stackp.tile([128, ND, T], BF16, tag="stk", name="stk")

        kvrem_cm = tc.tile_pool(name="kvrem", bufs=1)
        kvrem = kvrem_cm.__enter__()
        k_r = []
        v_r = []
        for rc in range(GROUP):
            kt_ = kvrem.tile([128, ND, T], BF16, tag=f"kr{rc}", name=f"kr{rc}")
            src = bass.AP(tensor=kv_all.ap().tensor, offset=rc * KVB,
                          ap=[[256, 128], [128 * 256, ND], [1, 256]])
            nc.scalar.dma_start(out=kt_.bitcast(F32), in_=src)
            k_r.append(kt_)
            vt_ = kvrem.tile([128, NT, VW], FP8, tag=f"vr{rc}",
                             name=f"vr{rc}")
            src = bass.AP(tensor=kv_all.ap().tensor,
                          offset=rc * KVB + KF // 2,
                          ap=[[VW // 4, 128], [128 * VW // 4, NT],
                              [1, VW // 4]])
            nc.scalar.dma_start(out=vt_.bitcast(F32), in_=src)
            v_r.append(vt_.rearrange("p t (h f) -> p t h f", h=H))

        with (
            tc.tile_pool(name="scps", bufs=2, space="PSUM") as scps,
            tc.tile_pool(name="pvps", bufs=1, space="PSUM") as pvps,
            tc.tile_pool(name="bcps", bufs=2, space="PSUM") as bcps,
            tc.tile_pool(name="scsb", bufs=3) as scsb,
            tc.tile_pool(name="prb", bufs=3) as prb,
            tc.tile_pool(name="accp", bufs=2) as accp,
            tc.tile_pool(name="tbp", bufs=2) as tbp,
        ):
            for d in range(ND):
                pvA = pvps.tile([65, T], F32, tag="pvA", name=f"pvA{d}")
                pvB = pvps.tile([65, T], F32, tag="pvB", name=f"pvB{d}")
                for half, pv in (("A", pvA), ("B", pvB)):
                    hoff = 0 if half == "A" else 64
                    hidx = 2 * d if half == "A" else 2 * d + 1
                    # software pipeline: scores/exp for pair p, PV for p-1
                    prs = [None] * NPAIR

                    def scores(p):
                        rc, pl = divmod(p, NT // 2)
                        ps = scps.tile([128, 2, T], F32, tag="scp",
                                       name=f"sc_{d}_{half}_{p}")
                        exf = scsb.tile([128, 2, T], F32, tag="exf",
                                        name=f"exf_{d}_{half}_{p}")
                        pr = prb.tile([128, 2, T], FP8, tag="prp",
                                      name=f"pr_{d}_{half}_{p}")
                        for j in range(2):
                            kt = pl * 2 + j
                            sl = slice(kt * 128, (kt + 1) * 128)
                            nc.tensor.matmul(
                                ps[:, j, :],
                                k_r[rc][hoff:hoff + 64, d, sl],
                                q_T[hoff:hoff + 64, d, :],
                                start=True, stop=True,
                                tile_position=(hoff, 0))
                            # ACT fp8 output costs ~2x: exp to f32 (fast),
                            # DVE does the fp8 downcast from SBUF.
                            nc.scalar.activation(
                                out=exf[:, j, :], in_=ps[:, j, :],
                                func=AF.Exp, scale=1.0 / np.sqrt(HD),
                                alpha=0.0)
                            nc.vector.tensor_copy(out=pr[:, j, :],
                                                  in_=exf[:, j, :])
                        prs[p] = pr

                    def pv_acc(p):
                        rc, pl = divmod(p, NT // 2)
                        vsl = v_r[rc][:, pl * 2:pl * 2 + 2, hidx, :]
                        nc.tensor.matmul(
                            pv, vsl, prs[p],
                            start=(p == 0), stop=(p == NPAIR - 1),
                            perf_mode=DR)

                    scores(0)
                    for p in range(NPAIR):
                        if p + 1 < NPAIR:
                            scores(p + 1)
                        pv_acc(p)

                # denominators: broadcast row 64, reciprocal, multiply
                for half, pv in (("A", pvA), ("B", pvB)):
                    acc = accp.tile([65, T], F32, tag="acc")
                    nc.vector.tensor_copy(out=acc.bitcast(F32R), in_=pv)
                    bc = bcps.tile([64, T], F32, tag="bc")
                    nc.tensor.matmul(bc, sel.bitcast(F32R), acc.bitcast(F32R),
                                     start=True, stop=True)
                    rec = tbp.tile([64, T], F32, tag="rec")
                    nc.vector.reciprocal(out=rec, in_=bc)
                    if half == "A":
                        nc.vector.tensor_mul(out=stacked[0:64, d, :],
                                             in0=acc[0:64, :], in1=rec)
                    else:
                        tmpB = tbp.tile([64, T], BF16, tag="tmpB")
                        nc.vector.tensor_mul(out=tmpB, in0=acc[0:64, :],
                                             in1=rec)
                        nc.sync.dma_start(out=stacked[64:128, d, :], in_=tmpB)

        kvrem_cm.__exit__(None, None, None)

        # ---- Phase F: out projection + residual + LN2, pipelined per t ---
        with (
            tc.tile_pool(name="wo", bufs=1) as wo,
            tc.tile_pool(name="ops", bufs=2, space="PSUM") as ops,
            tc.tile_pool(name="h2pool", bufs=2) as h2pool,
        ):
            wot = []
            for ch in range(2):
                w = wo.tile([128, ND, 512], BF16, tag=f"wot{ch}")
                nc.sync.dma_start(out=w, in_=wout_p.ap()[ch])
                wot.append(w)
            for t in range(NT):
                ps = [ops.tile([128, 512], F32, tag=f"ops{ch}",
                               name=f"ops_{ch}_{t}") for ch in range(2)]
                for d in range(ND):
                    for ch in range(2):
                        nc.tensor.matmul(
                            ps[ch], stacked[:, d, t * 128:(t + 1) * 128],
                            wot[ch][:, d, :],
                            start=(d == 0), stop=(d == ND - 1))
                for ch in range(2):
                    sl = slice(ch * 512, (ch + 1) * 512)
                    nc.vector.tensor_add(out=out1_N[t][:, sl],
                                         in0=x_N[t][:, sl], in1=ps[ch])
                    if "b_out" in bc_tiles:
                        nc.vector.tensor_add(out=out1_N[t][:, sl],
                                             in0=out1_N[t][:, sl],
                                             in1=bc_tiles["b_out"][:, sl])
                h2_N = h2pool.tile([128, D], BF16, tag="h2N")
                rstd, negmr = ln_stats(out1_N[t])
                nc.scalar.activation(out=h2_N, in_=out1_N[t], func=AF.Identity,
                                     scale=rstd, bias=negmr, alpha=0.0)
                if "ln2_g" in bc_tiles:
                    nc.vector.tensor_mul(out=h2_N, in0=h2_N,
                                         in1=bc_tiles["ln2_g"])
                    nc.vector.tensor_add(out=h2_N, in0=h2_N,
                                         in1=bc_tiles["ln2_b"])
                nc.scalar.dma_start_transpose(
                    out=h2_T[:, :, t * 128:(t + 1) * 128], in_=h2_N)

        stackp_cm.__exit__(None, None, None)

        # ---- Phase H: FFN1/FFN3 -> prod_T --------------------------------
        prp = stack.enter_context(tc.tile_pool(name="prp", bufs=1))
        prod_T = [prp.tile([128, T], BF16, tag=f"pr{h}", name=f"pr{h}")
                  for h in range(NH)]
        with (
            tc.tile_pool(name="wf", bufs=3) as wf,
            tc.tile_pool(name="ffps", bufs=2, space="PSUM") as ffps,
            tc.tile_pool(name="s1p", bufs=2) as s1p,
        ):
            for ht in range(NH):
                w1sb = wf.tile([128, ND, 128], BF16, tag="w1sb")
                w3sb = wf.tile([128, ND, 128], BF16, tag="w3sb")
                nc.sync.dma_start(out=w1sb, in_=w1_p.ap()[ht])
                nc.sync.dma_start(out=w3sb, in_=w3_p.ap()[ht])
                ps1 = ffps.tile([128, T], F32, tag="ps1")
                ps3 = ffps.tile([128, T], F32, tag="ps3")
                for d in range(ND):
                    nc.tensor.matmul(ps1, w1sb[:, d, :], h2_T[:, d, :],
                                     start=(d == 0), stop=(d == ND - 1))
                for d in range(ND):
                    nc.tensor.matmul(ps3, w3sb[:, d, :], h2_T[:, d, :],
                                     start=(d == 0), stop=(d == ND - 1))
                s1 = s1p.tile([128, T], BF16, tag="s1")
                b1arg = bc_tiles["b1"][:, ht:ht + 1] if "b1" in bc_tiles else 0.0
                nc.scalar.activation(out=s1, in_=ps1, func=AF.Silu,
                                     bias=b1arg, scale=1.0, alpha=0.0)
                t3 = s1p.tile([128, T], BF16, tag="t3")
                if "b3" in bc_tiles:
                    nc.vector.tensor_scalar_add(
                        out=t3, in0=ps3, scalar1=bc_tiles["b3"][:, ht:ht + 1])
                else:
                    nc.vector.tensor_copy(out=t3, in_=ps3)
                nc.vector.tensor_mul(out=prod_T[ht], in0=s1, in1=t3)

        # ---- Phase I: FFN2 + residual + store ----------------------------
        with (
            tc.tile_pool(name="w2p", bufs=2) as w2p,
            tc.tile_pool(name="f2ps", bufs=1, space="PSUM") as f2ps,
        ):
            ps = [[f2ps.tile([128, 512], F32, tag=f"f2_{t}_{ch}",
                             name=f"f2_{t}_{ch}") for ch in range(2)]
                  for t in range(NT)]
            for hg in range(NH // 8):
                w2g = w2p.tile([128, 8, D], BF16, tag="w2g")
                nc.sync.dma_start(out=w2g, in_=w2_p.ap()[hg])
                for hi in range(8):
                    ht = hg * 8 + hi
                    for t in range(NT):
                        for ch in range(2):
                            nc.tensor.matmul(
                                ps[t][ch],
                                prod_T[ht][:, t * 128:(t + 1) * 128],
                                w2g[:, hi, ch * 512:(ch + 1) * 512],
                                start=(ht == 0), stop=(ht == NH - 1))
            for t in range(NT):
                for ch in range(2):
                    sl = slice(ch * 512, (ch + 1) * 512)
                    nc.vector.tensor_add(out=out1_N[t][:, sl],
                                         in0=out1_N[t][:, sl], in1=ps[t][ch])
                    if "b2" in bc_tiles:
                        nc.vector.tensor_add(out=out1_N[t][:, sl],
                                             in0=out1_N[t][:, sl],
                                             in1=bc_tiles["b2"][:, sl])
                nc.sync.dma_start(out=out_p.ap()[t * 128:(t + 1) * 128, :],
                                  in_=out1_N[t])

    _split_all_waits(nc)
    return nc


# ---------------------------------------------------------------------------
# Host wrapper
# ---------------------------------------------------------------------------

_CACHE = {}
_PREP_CACHE = {}


def _prep_inputs(x, rope_cos, rope_sin, w_qkv, b_qkv, w_out, b_out,
                 qn_g, qn_b, kn_g, kn_b, ln1_g, ln1_b, ln2_g, ln2_b,
                 w1, b1, w2, b2, w3, b3):
    B, S, D = x.shape
    H, HD, FFN = 16, 64, 4096
    T = B * S // N_CORES
    ND, NH = D // 128, FFN // 128

    wkey = (id(w_qkv), id(w_out), id(w1), id(w2), id(w3),
            id(rope_cos), id(rope_sin))
    if wkey in _PREP_CACHE:
        shared, flags = _PREP_CACHE[wkey]
    else:
        flags = set()
        if not (np.all(ln1_g == 1) and np.all(ln1_b == 0)):
            flags.add("ln1_gb")
        if not (np.all(qn_g == 1) and np.all(qn_b == 0)):
            flags.add("qn_gb")
        if not (np.all(kn_g == 1) and np.all(kn_b == 0)):
            flags.add("kn_gb")
        if not (np.all(ln2_g == 1) and np.all(ln2_b == 0)):
            flags.add("ln2_gb")
        if np.any(b_qkv != 0):
            flags.add("bqkv")
        if np.any(b_out != 0):
            flags.add("bout")
        if np.any(b1 != 0):
            flags.add("b1")
        if np.any(b2 != 0):
            flags.add("b2")
        if np.any(b3 != 0):
            flags.add("b3")
        flags = frozenset(flags)

        bf = ml_dtypes.bfloat16
        # rope tables [S, D]: cos tiled over heads; sin with sign folded
        cosfull = np.tile(rope_cos, (1, H)).astype(bf)
        sinmod_half = np.concatenate(
            [-rope_sin[:, :HD // 2], rope_sin[:, HD // 2:]], axis=1)
        sinmod = np.tile(sinmod_half, (1, H)).astype(bf)

        wqkvT = np.ascontiguousarray(w_qkv.T)           # [D, 3D]
        # wqkv_t [6, 128, ND, 512]: storage order [k0,k1,v0,v1,q0,q1];
        # [ci, p, d, c] = wqkvT[d*128+p, ch*512+c]
        ch_order = [2, 3, 4, 5, 0, 1]
        wq4 = wqkvT.reshape(ND, 128, 6, 512)            # [d, p, ch, c]
        wqkv_t = np.ascontiguousarray(
            wq4.transpose(2, 1, 0, 3)[ch_order]).astype(bf)
        woutT = np.ascontiguousarray(w_out.T)           # [D, D]
        wo4 = woutT.reshape(ND, 128, 2, 512)
        wout_t = np.ascontiguousarray(wo4.transpose(2, 1, 0, 3)).astype(bf)
        # w1_t [NH, 128, ND, 128]: [ht, p, d, c] = w1[ht*128+c, d*128+p]
        w1r = w1.reshape(NH, 128, ND, 128)              # [ht, c, d, p]
        w1_t = np.ascontiguousarray(w1r.transpose(0, 3, 2, 1)).astype(bf)
        w3r = w3.reshape(NH, 128, ND, 128)
        w3_t = np.ascontiguousarray(w3r.transpose(0, 3, 2, 1)).astype(bf)
        # w2_t [NH//8, 128, 8, D]: [hg, p, hi, c] = w2[c, (hg*8+hi)*128+p]
        w2r = w2.reshape(D, NH // 8, 8, 128)            # [c, hg, hi, p]
        w2_t = np.ascontiguousarray(w2r.transpose(1, 3, 2, 0)).astype(bf)

        shared = {
            "wqkv_t": wqkv_t, "wout_t": wout_t,
            "w1_t": w1_t, "w3_t": w3_t, "w2_t": w2_t,
            "cosfull": cosfull, "sinmod": sinmod,
            # keep refs so ids stay unique
            "_refs": (w_qkv, w_out, w1, w2, w3, rope_cos, rope_sin),
        }
        opt = {"ln1_gb": [("ln1_g", ln1_g), ("ln1_b", ln1_b)],
               "qn_gb": [("qn_g", qn_g), ("qn_b", qn_b)],
               "kn_gb": [("kn_g", kn_g), ("kn_b", kn_b)],
               "ln2_gb": [("ln2_g", ln2_g), ("ln2_b", ln2_b)],
               "bqkv": [("b_qkv", b_qkv)], "bout": [("b_out", b_out)],
               "b1": [("b1", b1)], "b2": [("b2", b2)], "b3": [("b3", b3)]}
        for fl, items in opt.items():
            if fl in flags:
                for name, arr in items:
                    shared[name] = np.ascontiguousarray(arr).astype(np.float32)
        _PREP_CACHE[wkey] = (shared, flags)

    xf = np.ascontiguousarray(x.reshape(B * S, D)).astype(np.float32)
    in_maps = []
    for c in range(N_CORES):
        t0 = c * T
        m = {k: v for k, v in shared.items() if k != "_refs"}
        m["x"] = xf[t0:t0 + T]
        m["cosfull"] = shared["cosfull"][t0 % S:t0 % S + T]
        m["sinmod"] = shared["sinmod"][t0 % S:t0 % S + T]
        in_maps.append(m)
    return in_maps, flags, T, D


def kernel(**inputs):
    x = inputs["x"]
    B, S, D = x.shape
    in_maps, flags, T, _ = _prep_inputs(**inputs)

    key = (T, D, flags)
    if key not in _CACHE:
        _CACHE[key] = build_nc(T=T, D=D, flags=flags)
    nc = _CACHE[key]

    res = run_bass_kernel_spmd(nc, in_maps, core_ids=list(range(N_CORES)))
    out = np.empty((B * S, D), np.float32)
    for c in range(N_CORES):
        out[c * T:(c + 1) * T] = res.results[c]["out"]
    return out.reshape(B, S, D)


# revision 40
# speedup vs baseline: 1.0771x; 1.0200x over previous
"""Trainium2 Bass kernel for the pre-norm attention + SwiGLU FFN layer (v2).

Sharding: tokens (batch*seq flattened) split across 8 cores - 512 tokens
each; cores 0-3 hold batch 0, cores 4-7 batch 1. All per-token work (LNs,
projections, rope, FFN) is local with replicated weights; attention gathers
rope'd K (bf16) and ones-padded V (fp8e4m3) across each 4-core batch group
with two AllGathers (K first so scores can start while V is in flight),
then each core attends its 512 queries over the full 2048 context.

v2 vs v1:
 - all matmuls in bf16 (same PE rate as f32r, half the bytes); PV matmul in
   fp8 DoubleRow (2x PE rate; exp(scores) stays in [e^-5, e^4.6], inside
   e4m3 range, ones column exact).
 - transposes done by the DMA XBAR (dma_start_transpose, 2-byte dtype)
   instead of PE transposes + vector copy-backs.
 - weights are host-side pre-tiled so every weight DMA is contiguous per
   partition (2-16 KB lines instead of the 512 B packet storm).
 - denominators: ones-row PV output broadcast via one sel-matmul, one
   vector reciprocal on 64 partitions, one multiply (vs 32 single-partition
   reciprocals).
 - K/V projection and rope run before Q's so the collectives trigger early;
   Q-side prep overlaps the gathers.
 - attention inner loop is software-pipelined (scores for pair p+1 issued
   before the PV of pair p) to keep the PE p-state up.
"""

import numpy as np
import ml_dtypes

import bass_rust
import concourse.bass as bass
import concourse.mybir as mybir
import concourse.tile as tile
from concourse.bass_utils import run_bass_kernel_spmd
from concourse.vector_clock import ScopedClock

F32 = mybir.dt.float32
F32R = mybir.dt.float32r
BF16 = mybir.dt.bfloat16
FP8 = mybir.dt.float8e4
AF = mybir.ActivationFunctionType
DR = mybir.MatmulPerfMode.DoubleRow

N_CORES = 8
GROUP = 4
EPS = 1e-6

# ---------------------------------------------------------------------------
# Workaround for this walrus build's 1-wait-per-instruction encoding limit.
# ---------------------------------------------------------------------------
_MAX_WAITS = 1
_carrier_id = [0]


def _patched_drain_and_barrier(self, tick_clock, wait_clock):
    nc = self.nc
    drain_inst = nc.sync.drain()
    wait_clock.add_sem_waits(
        drain_inst.ins, ScopedClock({None: tick_clock.global_clock})
    )
    si = drain_inst.ins.sync_info
    waits = list(si.on_wait)
    if len(waits) > _MAX_WAITS:
        drain_inst.ins.sync_info = bass_rust.SyncInfo(
            on_wait=waits[:_MAX_WAITS], on_update=list(si.on_update)
        )
        rest = waits[_MAX_WAITS:]
        while rest:
            chunk, rest = rest[:_MAX_WAITS], rest[_MAX_WAITS:]
            extra = nc.sync.drain()
            extra.ins.sync_info = bass_rust.SyncInfo(on_wait=chunk, on_update=[])

    nc.all_engine_barrier()
    assert self.sems is not None
    popped = nc._tile_sem_poison_stack.pop()
    assert popped is self._sem_poison
    nc.clear_and_free_semaphores(list(self.sems.allocated().values()))
    nc.all_engine_barrier()


tile.TileContext._drain_and_barrier = _patched_drain_and_barrier


def _split_all_waits(nc, max_waits=_MAX_WAITS):
    for fn in nc.m.functions:
        for bb in fn.blocks:
            insts = list(bb.instructions)
            out = []
            changed = False
            for inst in insts:
                si = getattr(inst, "sync_info", None)
                if si is not None and si.on_wait and len(si.on_wait) > max_waits:
                    waits = list(si.on_wait)
                    updates = list(si.on_update)
                    extra, keep = waits[:-max_waits], waits[-max_waits:]
                    while extra:
                        chunk, extra = extra[:max_waits], extra[max_waits:]
                        _carrier_id[0] += 1
                        nop = mybir.InstNoOp(name=f"I-waitcar-{_carrier_id[0]}")
                        nop.engine = inst.engine
                        nop.sync_info = bass_rust.SyncInfo(on_wait=chunk, on_update=[])
                        nc.register_instruction(nop)
                        out.append(nop)
                    inst.sync_info = bass_rust.SyncInfo(on_wait=keep, on_update=updates)
                    changed = True
                out.append(inst)
            if changed:
                bb.instructions = out


# ---------------------------------------------------------------------------
# Graph builder (one SPMD program for all 8 cores)
# ---------------------------------------------------------------------------

def build_nc(T=512, D=1024, H=16, HD=64, FFN=4096, flags=frozenset()):
    NT = T // 128            # token tiles per core (4)
    ND = D // 128            # model-dim tiles (8)
    NH = FFN // 128          # ffn hidden tiles (32)
    D3 = 3 * D
    CTX = GROUP * T          # context tokens (2048)
    NKT = CTX // 128         # context k-token tiles (16)
    NPAIR = NKT // 2         # kt pairs for DoubleRow PV (8)
    VW = H * (HD + 1)        # padded v width per token (1040)
    KF = D * T               # k floats per rank
    VF = T * VW              # v elements per rank

    nc = bass.Bass(trn_type="TRN2", num_devices=N_CORES)

    x_p = nc.declare_dram_parameter("x", [T, D], F32, isOutput=False)
    cos_p = nc.declare_dram_parameter("cosfull", [T, D], BF16, isOutput=False)
    sin_p = nc.declare_dram_parameter("sinmod", [T, D], BF16, isOutput=False)
    # pre-tiled weights (see _prep_inputs for layouts)
    wqkv_p = nc.declare_dram_parameter("wqkv_t", [6, 128, ND, 512], BF16,
                                       isOutput=False)
    wout_p = nc.declare_dram_parameter("wout_t", [2, 128, ND, 512], BF16,
                                       isOutput=False)
    w1_p = nc.declare_dram_parameter("w1_t", [NH, 128, ND, 128], BF16,
                                     isOutput=False)
    w3_p = nc.declare_dram_parameter("w3_t", [NH, 128, ND, 128], BF16,
                                     isOutput=False)
    w2_p = nc.declare_dram_parameter("w2_t", [NH // 8, 128, 8, D], BF16,
                                     isOutput=False)
    vecs = {}
    for name, size in [("ln1_g", D), ("ln1_b", D), ("qn_g", D), ("qn_b", D),
                       ("kn_g", D), ("kn_b", D), ("ln2_g", D), ("ln2_b", D),
                       ("b_qkv", D3), ("b_out", D), ("b1", FFN), ("b3", FFN),
                       ("b2", D)]:
        flag = {"ln1_g": "ln1_gb", "ln1_b": "ln1_gb", "qn_g": "qn_gb",
                "qn_b": "qn_gb", "kn_g": "kn_gb", "kn_b": "kn_gb",
                "ln2_g": "ln2_gb", "ln2_b": "ln2_gb", "b_qkv": "bqkv",
                "b_out": "bout", "b1": "b1", "b3": "b3", "b2": "b2"}[name]
        if flag in flags:
            vecs[name] = nc.declare_dram_parameter(name, [size], F32,
                                                   isOutput=False)
    out_p = nc.declare_dram_parameter("out", [T, D], F32, isOutput=True)

    KVB = (2 * KF + VF) // 2  # bf16 elems per rank (k bf16 + v fp8, packed)
    kv_in = nc.dram_tensor("kv_in", [KVB], BF16)
    kv_all = nc.dram_tensor("kv_all", [GROUP * KVB], BF16)

    groups = [list(range(g * GROUP, (g + 1) * GROUP))
              for g in range(N_CORES // GROUP)]

    def bcast_ap(param, width):
        return bass.AP(tensor=param.ap().tensor, offset=0,
                       ap=[[0, 128], [1, width]])

    from contextlib import ExitStack
    with tile.TileContext(nc) as tc, ExitStack() as stack:
        const = stack.enter_context(tc.tile_pool(name="const", bufs=1))
        ones1 = const.tile([1, 64], BF16, tag="ones1")
        nc.vector.memset(ones1, 1.0)
        eps_t = const.tile([128, 1], F32, tag="eps")
        nc.vector.memset(eps_t, EPS)
        cosf = const.tile([128, NT, D], BF16, tag="cosf")
        sinm = const.tile([128, NT, D], BF16, tag="sinm")
        nc.sync.dma_start(
            out=cosf, in_=cos_p.ap().rearrange("(t p) d -> p t d", p=128))
        nc.sync.dma_start(
            out=sinm, in_=sin_p.ap().rearrange("(t p) d -> p t d", p=128))

        bc_tiles = {}
        for name in ("ln1_g", "ln1_b", "qn_g", "qn_b", "kn_g", "kn_b",
                     "ln2_g", "ln2_b", "b_out", "b2"):
            if name in vecs:
                t = const.tile([128, D], F32, tag=f"bc_{name}")
                nc.sync.dma_start(out=t, in_=bcast_ap(vecs[name], D))
                bc_tiles[name] = t
        if "b_qkv" in vecs:
            t = const.tile([128, D3], F32, tag="bc_bqkv")
            nc.sync.dma_start(out=t, in_=bcast_ap(vecs["b_qkv"], D3))
            bc_tiles["b_qkv"] = t
        for name in ("b1", "b3"):
            if name in vecs:
                t = const.tile([128, NH], F32, tag=f"col_{name}")
                ap = bass.AP(tensor=vecs[name].ap().tensor, offset=0,
                             ap=[[1, 128], [128, NH]])
                nc.sync.dma_start(out=t, in_=ap)
                bc_tiles[name] = t

        stat = stack.enter_context(tc.tile_pool(name="stat", bufs=4))
        xres = stack.enter_context(tc.tile_pool(name="xres", bufs=1))
        o1p = stack.enter_context(tc.tile_pool(name="o1p", bufs=1))

        x_N = [xres.tile([128, D], F32, tag=f"x{t}", name=f"x{t}")
               for t in range(NT)]
        out1_N = [o1p.tile([128, D], F32, tag=f"o1{t}", name=f"o1{t}")
                  for t in range(NT)]
        qkT_pool = stack.enter_context(tc.tile_pool(name="qkTp", bufs=1))
        q_T = qkT_pool.tile([128, ND, T], BF16, tag="qT", name="qT")
        k_T = qkT_pool.tile([128, ND, T], BF16, tag="kT", name="kT")

        def ln_stats(src_tile):
            """rstd [128,1], negm_r [128,1] for LN over D free elems."""
            st = stat.tile([128, 2, 6], F32, tag="lnst")
            nc.vector.bn_stats(out=st[:, 0, :], in_=src_tile[:, 0:D // 2])
            nc.vector.bn_stats(out=st[:, 1, :], in_=src_tile[:, D // 2:D])
            mv = stat.tile([128, 2], F32, tag="lnmv")
            nc.vector.bn_aggr(out=mv, in_=st)
            rstd = stat.tile([128, 1], F32, tag="lnrstd")
            nc.scalar.activation(out=rstd, in_=mv[:, 1:2], func=AF.Sqrt,
                                 bias=eps_t, scale=1.0, alpha=0.0)
            nc.vector.reciprocal(out=rstd, in_=rstd)
            negmr = stat.tile([128, 1], F32, tag="lnnm")
            nc.vector.tensor_mul(out=negmr, in0=mv[:, 0:1], in1=rstd)
            nc.scalar.mul(out=negmr, in_=negmr, mul=-1.0)
            return rstd, negmr

        # ---- Phase A: load x, LN1 -> h (bf16), DMA-transpose -> h_T ------
        hTp = tc.tile_pool(name="hTp", bufs=1)
        hT_pool = hTp.__enter__()
        h_T = hT_pool.tile([128, ND, T], BF16, tag="hT", name="hT")
        with tc.tile_pool(name="hpool", bufs=2) as hpool:
            for t in range(NT):
                nc.sync.dma_start(out=x_N[t],
                                  in_=x_p.ap()[t * 128:(t + 1) * 128, :])
                h_N = hpool.tile([128, D], BF16, tag="hN")
                rstd, negmr = ln_stats(x_N[t])
                nc.scalar.activation(out=h_N, in_=x_N[t], func=AF.Identity,
                                     scale=rstd, bias=negmr, alpha=0.0)
                if "ln1_g" in bc_tiles:
                    nc.vector.tensor_mul(out=h_N, in0=h_N,
                                         in1=bc_tiles["ln1_g"])
                    nc.vector.tensor_add(out=h_N, in0=h_N,
                                         in1=bc_tiles["ln1_b"])
                nc.scalar.dma_start_transpose(
                    out=h_T[:, :, t * 128:(t + 1) * 128], in_=h_N)

        # ---- Phase B/C: QKV projection, K/V first, gathers early ---------
        # wqkv_t storage order: [k0, k1, v0, v1, q0, q1]
        qknp_cm = tc.tile_pool(name="qknp", bufs=1)
        qknp = qknp_cm.__enter__()
        q_N = [qknp.tile([128, D], BF16, tag=f"qN{t}", name=f"qN{t}")
               for t in range(NT)]
        k_N = [qknp.tile([128, D], BF16, tag=f"kN{t}", name=f"kN{t}")
               for t in range(NT)]
        vpp = tc.tile_pool(name="vpp", bufs=1)
        vp_pool = vpp.__enter__()
        v_flat = vp_pool.tile([128, NT, VW], FP8, tag="vpad", name="vpad")
        v_pad = v_flat.rearrange("p t (h f) -> p t h f", h=H)
        nc.vector.memset(v_pad[:, :, :, HD:HD + 1], 1.0)

        wq_cm = tc.tile_pool(name="wq", bufs=2)
        wq_pool = wq_cm.__enter__()
        mmps_cm = tc.tile_pool(name="mmps", bufs=2, space="PSUM")
        mmps = mmps_cm.__enter__()

        def qkv_chunk(ci):
            """ci: storage index into wqkv_t [k0,k1,v0,v1,q0,q1]."""
            w = wq_pool.tile([128, ND, 512], BF16, tag="wqt")
            nc.sync.dma_start(out=w, in_=wqkv_p.ap()[ci])
            ps = [mmps.tile([128, 512], F32, tag=f"qkvps{t}",
                            name=f"qkvps_{ci}_{t}") for t in range(NT)]
            for d in range(ND):
                for t in range(NT):
                    nc.tensor.matmul(
                        ps[t], h_T[:, d, t * 128:(t + 1) * 128], w[:, d, :],
                        start=(d == 0), stop=(d == ND - 1))
            # logical chunk: 0,1=q; 2,3=k; 4,5=v
            ch = [2, 3, 4, 5, 0, 1][ci]
            for t in range(NT):
                if ch < 2:
                    dst = q_N[t][:, (ch % 2) * 512:(ch % 2) * 512 + 512]
                elif ch < 4:
                    dst = k_N[t][:, (ch % 2) * 512:(ch % 2) * 512 + 512]
                else:
                    h0 = (ch - 4) * 8
                    dst = v_pad[:, t, h0:h0 + 8, 0:HD]
                    if "b_qkv" in bc_tiles:
                        nc.vector.tensor_add(
                            out=dst,
                            in0=bc_tiles["b_qkv"][:, ch * 512:(ch + 1) * 512]
                            .rearrange("p (h f) -> p h f", h=8),
                            in1=ps[t].rearrange("p (h f) -> p h f", h=8))
                    else:
                        nc.vector.tensor_copy(
                            out=dst,
                            in_=ps[t].rearrange("p (h f) -> p h f", h=8))
                    continue
                if "b_qkv" in bc_tiles:
                    nc.vector.tensor_add(
                        out=dst,
                        in0=bc_tiles["b_qkv"][:, ch * 512:(ch + 1) * 512],
                        in1=ps[t])
                else:
                    nc.vector.tensor_copy(out=dst, in_=ps[t])

        def qknorm_rope(src_N, dst_T, gname, t):
            """qk-norm + rope on [128, D] bf16, DMA-transpose into dst_T."""
            rstd, negmr = ln_stats(src_N)
            nrm = rope_pool.tile([128, D], BF16, tag="nrm")
            nc.scalar.activation(out=nrm, in_=src_N, func=AF.Identity,
                                 scale=rstd, bias=negmr, alpha=0.0)
            if f"{gname}_g" in bc_tiles:
                nc.vector.tensor_mul(out=nrm, in0=nrm,
                                     in1=bc_tiles[f"{gname}_g"])
                nc.vector.tensor_add(out=nrm, in0=nrm,
                                     in1=bc_tiles[f"{gname}_b"])
            nrm3 = nrm.rearrange("p (h f) -> p h f", h=H)
            sw = rope_pool.tile([128, H, HD], BF16, tag="sw")
            nc.vector.tensor_copy(out=sw[:, :, 0:32], in_=nrm3[:, :, 32:64])
            nc.vector.tensor_copy(out=sw[:, :, 32:64], in_=nrm3[:, :, 0:32])
            swf = sw.rearrange("p h f -> p (h f)")
            rp = rope_pool.tile([128, D], BF16, tag="rp")
            nc.vector.tensor_mul(out=rp, in0=nrm, in1=cosf[:, t, :])
            nc.vector.tensor_mul(out=swf, in0=swf, in1=sinm[:, t, :])
            nc.vector.tensor_add(out=rp, in0=rp, in1=swf)
            nc.scalar.dma_start_transpose(
                out=dst_T[:, :, t * 128:(t + 1) * 128], in_=rp)

        rope_cm = tc.tile_pool(name="ropep", bufs=2)
        rope_pool = rope_cm.__enter__()

        # K chunks -> k norm/rope -> bounce; V chunks -> bounce; one gather
        qkv_chunk(0)
        qkv_chunk(1)
        for t in range(NT):
            qknorm_rope(k_N[t], k_T, "kn", t)
        k_dst = bass.AP(tensor=kv_in.ap().tensor, offset=0,
                        ap=[[512, 128], [128 * 512, ND], [1, 512]])
        nc.scalar.dma_start(out=k_dst, in_=k_T)

        qkv_chunk(2)
        qkv_chunk(3)
        v_dst = bass.AP(tensor=kv_in.ap().tensor, offset=KF,
                        ap=[[VW // 2, 128], [128 * VW // 2, NT], [1, VW // 2]])
        nc.scalar.dma_start(out=v_dst, in_=v_flat.bitcast(BF16))
        nc.gpsimd.collective_compute(
            "AllGather", mybir.AluOpType.bypass, replica_groups=groups,
            ins=[kv_in.ap().opt()], outs=[kv_all.ap().opt()])

        # Q chunks + q norm/rope (overlaps the gather)
        qkv_chunk(4)
        qkv_chunk(5)
        for t in range(NT):
            qknorm_rope(q_N[t], q_T, "qn", t)

        rope_cm.__exit__(None, None, None)
        mmps_cm.__exit__(None, None, None)
        wq_cm.__exit__(None, None, None)
        vpp.__exit__(None, None, None)
        qknp_cm.__exit__(None, None, None)
        hTp.__exit__(None, None, None)

        # ---- Phase E: attention ------------------------------------------
        h2Tp = stack.enter_context(tc.tile_pool(name="h2Tp", bufs=1))
        h2_T = h2Tp.tile([128, ND, T], BF16, tag="h2T", name="h2T")
        stackp_cm = tc.tile_pool(name="stackp", bufs=1)
        stackp = stackp_cm.__enter__()
        stacked = stackp.tile([128, ND, T], BF16, tag="stk", name="stk")

        kvrem_cm = tc.tile_pool(name="kvrem", bufs=1)
        kvrem = kvrem_cm.__enter__()
        k_r = []
        v_r = []
        for rc in range(GROUP):
            kt_ = kvrem.tile([128, ND, T], BF16, tag=f"kr{rc}", name=f"kr{rc}")
            src = bass.AP(tensor=kv_all.ap().tensor, offset=rc * KVB,
                          ap=[[512, 128], [128 * 512, ND], [1, 512]])
            nc.scalar.dma_start(out=kt_, in_=src)
            k_r.append(kt_)
            vt_ = kvrem.tile([128, NT, VW], FP8, tag=f"vr{rc}",
                             name=f"vr{rc}")
            src = bass.AP(tensor=kv_all.ap().tensor,
                          offset=rc * KVB + KF,
                          ap=[[VW // 2, 128], [128 * VW // 2, NT],
                              [1, VW // 2]])
            nc.scalar.dma_start(out=vt_.bitcast(BF16), in_=src)
            v_r.append(vt_.rearrange("p t (h f) -> p t h f", h=H))

        with (
            tc.tile_pool(name="scps", bufs=5, space="PSUM") as scps,
            tc.tile_pool(name="pvps", bufs=1, space="PSUM") as pvps,
            tc.tile_pool(name="bcps", bufs=1, space="PSUM") as bcps,
            tc.tile_pool(name="prb", bufs=4) as prb,
            tc.tile_pool(name="accp", bufs=2) as accp,
            tc.tile_pool(name="tbp", bufs=2) as tbp,
        ):
            for d in range(ND):
                pvA = pvps.tile([65, T], F32, tag="pvA", name=f"pvA{d}")
                pvB = pvps.tile([65, T], F32, tag="pvB", name=f"pvB{d}")
                for half, pv in (("A", pvA), ("B", pvB)):
                    hoff = 0 if half == "A" else 64
                    hidx = 2 * d if half == "A" else 2 * d + 1
                    # software pipeline: scores/exp for pair p+1 and p+2
                    # issued before the PV of pair p (keeps PE and ACT busy)
                    prs = [None] * NPAIR

                    def scores(p):
                        rc, pl = divmod(p, NT // 2)
                        pr = prb.tile([128, 2, T], FP8, tag="prp",
                                      name=f"pr_{d}_{half}_{p}")
                        for j in range(2):
                            kt = pl * 2 + j
                            sl = slice(kt * 128, (kt + 1) * 128)
                            ps = scps.tile([128, T], F32, tag="scp",
                                           name=f"sc_{d}_{half}_{p}_{j}")
                            nc.tensor.matmul(
                                ps,
                                k_r[rc][hoff:hoff + 64, d, sl],
                                q_T[hoff:hoff + 64, d, :],
                                start=True, stop=True,
                                tile_position=(hoff, 0))
                            nc.scalar.activation(
                                out=pr[:, j, :], in_=ps,
                                func=AF.Exp, scale=1.0 / np.sqrt(HD),
                                alpha=0.0)
                        prs[p] = pr

                    def pv_acc(p):
                        rc, pl = divmod(p, NT // 2)
                        vsl = v_r[rc][:, pl * 2:pl * 2 + 2, hidx, :]
                        nc.tensor.matmul(
                            pv, vsl, prs[p],
                            start=(p == 0), stop=(p == NPAIR - 1),
                            perf_mode=DR)

                    scores(0)
                    scores(1)
                    for p in range(NPAIR):
                        if p + 2 < NPAIR:
                            scores(p + 2)
                        pv_acc(p)

                # denominators: reciprocal of the ones-row, K=1 matmul
                # broadcast across 64 partitions, multiply
                for half, pv in (("A", pvA), ("B", pvB)):
                    acc = accp.tile([65, T], F32, tag="acc")
                    nc.vector.tensor_copy(out=acc.bitcast(F32R), in_=pv)
                    rrow = tbp.tile([1, T], BF16, tag="rrow")
                    with nc.allow_low_precision(reason="bf16 denominators"):
                        nc.vector.reciprocal(out=rrow, in_=acc[64:65, :])
                    bc = bcps.tile([64, T], F32, tag="bc")
                    nc.tensor.matmul(bc, ones1, rrow, start=True, stop=True)
                    if half == "A":
                        nc.vector.tensor_mul(out=stacked[0:64, d, :],
                                             in0=acc[0:64, :], in1=bc)
                    else:
                        tmpB = tbp.tile([64, T], BF16, tag="tmpB")
                        nc.vector.tensor_mul(out=tmpB, in0=acc[0:64, :],
                                             in1=bc)
                        nc.sync.dma_start(out=stacked[64:128, d, :], in_=tmpB)

        kvrem_cm.__exit__(None, None, None)

        # ---- Phase F: out projection + residual + LN2, pipelined per t ---
        with (
            tc.tile_pool(name="wo", bufs=1) as wo,
            tc.tile_pool(name="ops", bufs=2, space="PSUM") as ops,
            tc.tile_pool(name="h2pool", bufs=2) as h2pool,
        ):
            wot = []
            for ch in range(2):
                w = wo.tile([128, ND, 512], BF16, tag=f"wot{ch}")
                nc.sync.dma_start(out=w, in_=wout_p.ap()[ch])
                wot.append(w)
            for t in range(NT):
                ps = [ops.tile([128, 512], F32, tag=f"ops{ch}",
                               name=f"ops_{ch}_{t}") for ch in range(2)]
                for d in range(ND):
                    for ch in range(2):
                        nc.tensor.matmul(
                            ps[ch], stacked[:, d, t * 128:(t + 1) * 128],
                            wot[ch][:, d, :],
                            start=(d == 0), stop=(d == ND - 1))
                for ch in range(2):
                    sl = slice(ch * 512, (ch + 1) * 512)
                    nc.vector.tensor_add(out=out1_N[t][:, sl],
                                         in0=x_N[t][:, sl], in1=ps[ch])
                    if "b_out" in bc_tiles:
                        nc.vector.tensor_add(out=out1_N[t][:, sl],
                                             in0=out1_N[t][:, sl],
                                             in1=bc_tiles["b_out"][:, sl])
                h2_N = h2pool.tile([128, D], BF16, tag="h2N")
                rstd, negmr = ln_stats(out1_N[t])
                nc.scalar.activation(out=h2_N, in_=out1_N[t], func=AF.Identity,
                                     scale=rstd, bias=negmr, alpha=0.0)
                if "ln2_g" in bc_tiles:
                    nc.vector.tensor_mul(out=h2_N, in0=h2_N,
                                         in1=bc_tiles["ln2_g"])
                    nc.vector.tensor_add(out=h2_N, in0=h2_N,
                                         in1=bc_tiles["ln2_b"])
                nc.scalar.dma_start_transpose(
                    out=h2_T[:, :, t * 128:(t + 1) * 128], in_=h2_N)

        stackp_cm.__exit__(None, None, None)

        # ---- Phase H: FFN1/FFN3 -> prod_T --------------------------------
        prp = stack.enter_context(tc.tile_pool(name="prp", bufs=1))
        prod_T = [prp.tile([128, T], BF16, tag=f"pr{h}", name=f"pr{h}")
                  for h in range(NH)]
        with (
            tc.tile_pool(name="wf", bufs=3) as wf,
            tc.tile_pool(name="ffps", bufs=2, space="PSUM") as ffps,
            tc.tile_pool(name="s1p", bufs=2) as s1p,
        ):
            for ht in range(NH):
                w1sb = wf.tile([128, ND, 128], BF16, tag="w1sb")
                w3sb = wf.tile([128, ND, 128], BF16, tag="w3sb")
                nc.sync.dma_start(out=w1sb, in_=w1_p.ap()[ht])
                nc.sync.dma_start(out=w3sb, in_=w3_p.ap()[ht])
                ps1 = ffps.tile([128, T], F32, tag="ps1")
                ps3 = ffps.tile([128, T], F32, tag="ps3")
                for d in range(ND):
                    nc.tensor.matmul(ps1, w1sb[:, d, :], h2_T[:, d, :],
                                     start=(d == 0), stop=(d == ND - 1))
                for d in range(ND):
                    nc.tensor.matmul(ps3, w3sb[:, d, :], h2_T[:, d, :],
                                     start=(d == 0), stop=(d == ND - 1))
                s1 = s1p.tile([128, T], BF16, tag="s1")
                b1arg = bc_tiles["b1"][:, ht:ht + 1] if "b1" in bc_tiles else 0.0
                nc.scalar.activation(out=s1, in_=ps1, func=AF.Silu,
                                     bias=b1arg, scale=1.0, alpha=0.0)
                t3 = s1p.tile([128, T], BF16, tag="t3")
                if "b3" in bc_tiles:
                    nc.vector.tensor_scalar_add(
                        out=t3, in0=ps3, scalar1=bc_tiles["b3"][:, ht:ht + 1])
                else:
                    nc.vector.tensor_copy(out=t3, in_=ps3)
                nc.vector.tensor_mul(out=prod_T[ht], in0=s1, in1=t3)

        # ---- Phase I: FFN2 + residual + store ----------------------------
        with (
            tc.tile_pool(name="w2p", bufs=2) as w2p,
            tc.tile_pool(name="f2ps", bufs=1, space="PSUM") as f2ps,
        ):
            ps = [[f2ps.tile([128, 512], F32, tag=f"f2_{t}_{ch}",
                             name=f"f2_{t}_{ch}") for ch in range(2)]
                  for t in range(NT)]
            for hg in range(NH // 8):
                w2g = w2p.tile([128, 8, D], BF16, tag="w2g")
                nc.sync.dma_start(out=w2g, in_=w2_p.ap()[hg])
                for hi in range(8):
                    ht = hg * 8 + hi
                    for t in range(NT):
                        for ch in range(2):
                            nc.tensor.matmul(
                                ps[t][ch],
                                prod_T[ht][:, t * 128:(t + 1) * 128],
                                w2g[:, hi, ch * 512:(ch + 1) * 512],
                                start=(ht == 0), stop=(ht == NH - 1))
            for t in range(NT):
                for ch in range(2):
                    sl = slice(ch * 512, (ch + 1) * 512)
                    nc.vector.tensor_add(out=out1_N[t][:, sl],
                                         in0=out1_N[t][:, sl], in1=ps[t][ch])
                    if "b2" in bc_tiles:
                        nc.vector.tensor_add(out=out1_N[t][:, sl],
                                             in0=out1_N[t][:, sl],
                                             in1=bc_tiles["b2"][:, sl])
                nc.sync.dma_start(out=out_p.ap()[t * 128:(t + 1) * 128, :],
                                  in_=out1_N[t])

    _split_all_waits(nc)
    return nc


# ---------------------------------------------------------------------------
# Host wrapper
# ---------------------------------------------------------------------------

_CACHE = {}
_PREP_CACHE = {}


def _prep_inputs(x, rope_cos, rope_sin, w_qkv, b_qkv, w_out, b_out,
                 qn_g, qn_b, kn_g, kn_b, ln1_g, ln1_b, ln2_g, ln2_b,
                 w1, b1, w2, b2, w3, b3):
    B, S, D = x.shape
    H, HD, FFN = 16, 64, 4096
    T = B * S // N_CORES
    ND, NH = D // 128, FFN // 128

    wkey = (id(w_qkv), id(w_out), id(w1), id(w2), id(w3),
            id(rope_cos), id(rope_sin))
    if wkey in _PREP_CACHE:
        shared, flags = _PREP_CACHE[wkey]
    else:
        flags = set()
        if not (np.all(ln1_g == 1) and np.all(ln1_b == 0)):
            flags.add("ln1_gb")
        if not (np.all(qn_g == 1) and np.all(qn_b == 0)):
            flags.add("qn_gb")
        if not (np.all(kn_g == 1) and np.all(kn_b == 0)):
            flags.add("kn_gb")
        if not (np.all(ln2_g == 1) and np.all(ln2_b == 0)):
            flags.add("ln2_gb")
        if np.any(b_qkv != 0):
            flags.add("bqkv")
        if np.any(b_out != 0):
            flags.add("bout")
        if np.any(b1 != 0):
            flags.add("b1")
        if np.any(b2 != 0):
            flags.add("b2")
        if np.any(b3 != 0):
            flags.add("b3")
        flags = frozenset(flags)

        bf = ml_dtypes.bfloat16
        # rope tables [S, D]: cos tiled over heads; sin with sign folded
        cosfull = np.tile(rope_cos, (1, H)).astype(bf)
        sinmod_half = np.concatenate(
            [-rope_sin[:, :HD // 2], rope_sin[:, HD // 2:]], axis=1)
        sinmod = np.tile(sinmod_half, (1, H)).astype(bf)

        wqkvT = np.ascontiguousarray(w_qkv.T)           # [D, 3D]
        # wqkv_t [6, 128, ND, 512]: storage order [k0,k1,v0,v1,q0,q1];
        # [ci, p, d, c] = wqkvT[d*128+p, ch*512+c]
        ch_order = [2, 3, 4, 5, 0, 1]
        wq4 = wqkvT.reshape(ND, 128, 6, 512)            # [d, p, ch, c]
        wqkv_t = np.ascontiguousarray(
            wq4.transpose(2, 1, 0, 3)[ch_order]).astype(bf)
        woutT = np.ascontiguousarray(w_out.T)           # [D, D]
        wo4 = woutT.reshape(ND, 128, 2, 512)
        wout_t = np.ascontiguousarray(wo4.transpose(2, 1, 0, 3)).astype(bf)
        # w1_t [NH, 128, ND, 128]: [ht, p, d, c] = w1[ht*128+c, d*128+p]
        w1r = w1.reshape(NH, 128, ND, 128)              # [ht, c, d, p]
        w1_t = np.ascontiguousarray(w1r.transpose(0, 3, 2, 1)).astype(bf)
        w3r = w3.reshape(NH, 128, ND, 128)
        w3_t = np.ascontiguousarray(w3r.transpose(0, 3, 2, 1)).astype(bf)
        # w2_t [NH//8, 128, 8, D]: [hg, p, hi, c] = w2[c, (hg*8+hi)*128+p]
        w2r = w2.reshape(D, NH // 8, 8, 128)            # [c, hg, hi, p]
        w2_t = np.ascontiguousarray(w2r.transpose(1, 3, 2, 0)).astype(bf)

        shared = {
            "wqkv_t": wqkv_t, "wout_t": wout_t,
            "w1_t": w1_t, "w3_t": w3_t, "w2_t": w2_t,
            "cosfull": cosfull, "sinmod": sinmod,
            # keep refs so ids stay unique
            "_refs": (w_qkv, w_out, w1, w2, w3, rope_cos, rope_sin),
        }
        opt = {"ln1_gb": [("ln1_g", ln1_g), ("ln1_b", ln1_b)],
               "qn_gb": [("qn_g", qn_g), ("qn_b", qn_b)],
               "kn_gb": [("kn_g", kn_g), ("kn_b", kn_b)],
               "ln2_gb": [("ln2_g", ln2_g), ("ln2_b", ln2_b)],
               "bqkv": [("b_qkv", b_qkv)], "bout": [("b_out", b_out)],
               "b1": [("b1", b1)], "b2": [("b2", b2)], "b3": [("b3", b3)]}
        for fl, items in opt.items():
            if fl in flags:
                for name, arr in items:
                    shared[name] = np.ascontiguousarray(arr).astype(np.float32)
        _PREP_CACHE[wkey] = (shared, flags)

    xf = np.ascontiguousarray(x.reshape(B * S, D)).astype(np.float32)
    in_maps = []
    for c in range(N_CORES):
        t0 = c * T
        m = {k: v for k, v in shared.items() if k != "_refs"}
        m["x"] = xf[t0:t0 + T]
        m["cosfull"] = shared["cosfull"][t0 % S:t0 % S + T]
        m["sinmod"] = shared["sinmod"][t0 % S:t0 % S + T]
        in_maps.append(m)
    return in_maps, flags, T, D


def kernel(**inputs):
    x = inputs["x"]
    B, S, D = x.shape
    in_maps, flags, T, _ = _prep_inputs(**inputs)

    key = (T, D, flags)
    if key not in _CACHE:
        _CACHE[key] = build_nc(T=T, D=D, flags=flags)
    nc = _CACHE[key]

    res = run_bass_kernel_spmd(nc, in_maps, core_ids=list(range(N_CORES)))
    out = np.empty((B * S, D), np.float32)
    for c in range(N_CORES):
        out[c * T:(c + 1) * T] = res.results[c]["out"]
    return out.reshape(B, S, D)


# revision 47
# speedup vs baseline: 1.1442x; 1.0623x over previous
"""Trainium2 Bass kernel for the pre-norm attention + SwiGLU FFN layer (v2).

Sharding: tokens (batch*seq flattened) split across 8 cores - 512 tokens
each; cores 0-3 hold batch 0, cores 4-7 batch 1. All per-token work (LNs,
projections, rope, FFN) is local with replicated weights; attention gathers
rope'd K (bf16) and ones-padded V (fp8e4m3) across each 4-core batch group
with two AllGathers (K first so scores can start while V is in flight),
then each core attends its 512 queries over the full 2048 context.

v2 vs v1:
 - all matmuls in bf16 (same PE rate as f32r, half the bytes); PV matmul in
   fp8 DoubleRow (2x PE rate; exp(scores) stays in [e^-5, e^4.6], inside
   e4m3 range, ones column exact).
 - transposes done by the DMA XBAR (dma_start_transpose, 2-byte dtype)
   instead of PE transposes + vector copy-backs.
 - weights are host-side pre-tiled so every weight DMA is contiguous per
   partition (2-16 KB lines instead of the 512 B packet storm).
 - denominators: ones-row PV output broadcast via one sel-matmul, one
   vector reciprocal on 64 partitions, one multiply (vs 32 single-partition
   reciprocals).
 - K/V projection and rope run before Q's so the collectives trigger early;
   Q-side prep overlaps the gathers.
 - attention inner loop is software-pipelined (scores for pair p+1 issued
   before the PV of pair p) to keep the PE p-state up.
"""

import numpy as np
import ml_dtypes

import bass_rust
import concourse.bass as bass
import concourse.mybir as mybir
import concourse.tile as tile
from concourse.bass_utils import run_bass_kernel_spmd
from concourse.vector_clock import ScopedClock

F32 = mybir.dt.float32
F32R = mybir.dt.float32r
BF16 = mybir.dt.bfloat16
FP8 = mybir.dt.float8e4
AF = mybir.ActivationFunctionType
DR = mybir.MatmulPerfMode.DoubleRow

N_CORES = 8
GROUP = 4
EPS = 1e-6

# ---------------------------------------------------------------------------
# Workaround for this walrus build's 1-wait-per-instruction encoding limit.
# ---------------------------------------------------------------------------
_MAX_WAITS = 1
_carrier_id = [0]


def _patched_drain_and_barrier(self, tick_clock, wait_clock):
    nc = self.nc
    drain_inst = nc.sync.drain()
    wait_clock.add_sem_waits(
        drain_inst.ins, ScopedClock({None: tick_clock.global_clock})
    )
    si = drain_inst.ins.sync_info
    waits = list(si.on_wait)
    if len(waits) > _MAX_WAITS:
        drain_inst.ins.sync_info = bass_rust.SyncInfo(
            on_wait=waits[:_MAX_WAITS], on_update=list(si.on_update)
        )
        rest = waits[_MAX_WAITS:]
        while rest:
            chunk, rest = rest[:_MAX_WAITS], rest[_MAX_WAITS:]
            extra = nc.sync.drain()
            extra.ins.sync_info = bass_rust.SyncInfo(on_wait=chunk, on_update=[])

    nc.all_engine_barrier()
    assert self.sems is not None
    popped = nc._tile_sem_poison_stack.pop()
    assert popped is self._sem_poison
    nc.clear_and_free_semaphores(list(self.sems.allocated().values()))
    nc.all_engine_barrier()


tile.TileContext._drain_and_barrier = _patched_drain_and_barrier


def _split_all_waits(nc, max_waits=_MAX_WAITS):
    for fn in nc.m.functions:
        for bb in fn.blocks:
            insts = list(bb.instructions)
            out = []
            changed = False
            for inst in insts:
                si = getattr(inst, "sync_info", None)
                if si is not None and si.on_wait and len(si.on_wait) > max_waits:
                    waits = list(si.on_wait)
                    updates = list(si.on_update)
                    extra, keep = waits[:-max_waits], waits[-max_waits:]
                    while extra:
                        chunk, extra = extra[:max_waits], extra[max_waits:]
                        _carrier_id[0] += 1
                        nop = mybir.InstNoOp(name=f"I-waitcar-{_carrier_id[0]}")
                        nop.engine = inst.engine
                        nop.sync_info = bass_rust.SyncInfo(on_wait=chunk, on_update=[])
                        nc.register_instruction(nop)
                        out.append(nop)
                    inst.sync_info = bass_rust.SyncInfo(on_wait=keep, on_update=updates)
                    changed = True
                out.append(inst)
            if changed:
                bb.instructions = out


# ---------------------------------------------------------------------------
# Graph builder (one SPMD program for all 8 cores)
# ---------------------------------------------------------------------------

def build_nc(T=512, D=1024, H=16, HD=64, FFN=4096, flags=frozenset()):
    NT = T // 128            # token tiles per core (4)
    ND = D // 128            # model-dim tiles (8)
    NH = FFN // 128          # ffn hidden tiles (32)
    D3 = 3 * D
    CTX = GROUP * T          # context tokens (2048)
    NKT = CTX // 128         # context k-token tiles (16)
    NPAIR = NKT // 2         # kt pairs for DoubleRow PV (8)
    VW = H * (HD + 1)        # padded v width per token (1040)
    KF = D * T               # k floats per rank
    VF = T * VW              # v elements per rank

    nc = bass.Bass(trn_type="TRN2", num_devices=N_CORES)

    x_p = nc.declare_dram_parameter("x", [T, D], F32, isOutput=False)
    cos_p = nc.declare_dram_parameter("cosfull", [T, D], BF16, isOutput=False)
    sin_p = nc.declare_dram_parameter("sinmod", [T, D], BF16, isOutput=False)
    # pre-tiled weights (see _prep_inputs for layouts)
    wqkv_p = nc.declare_dram_parameter("wqkv_t", [6, 128, ND, 512], BF16,
                                       isOutput=False)
    wout_p = nc.declare_dram_parameter("wout_t", [2, 128, ND, 512], BF16,
                                       isOutput=False)
    w1_p = nc.declare_dram_parameter("w1_t", [NH, 128, ND, 128], BF16,
                                     isOutput=False)
    w3_p = nc.declare_dram_parameter("w3_t", [NH, 128, ND, 128], BF16,
                                     isOutput=False)
    w2_p = nc.declare_dram_parameter("w2_t", [NH // 8, 128, 8, D], BF16,
                                     isOutput=False)
    vecs = {}
    for name, size in [("ln1_g", D), ("ln1_b", D), ("qn_g", D), ("qn_b", D),
                       ("kn_g", D), ("kn_b", D), ("ln2_g", D), ("ln2_b", D),
                       ("b_qkv", D3), ("b_out", D), ("b1", FFN), ("b3", FFN),
                       ("b2", D)]:
        flag = {"ln1_g": "ln1_gb", "ln1_b": "ln1_gb", "qn_g": "qn_gb",
                "qn_b": "qn_gb", "kn_g": "kn_gb", "kn_b": "kn_gb",
                "ln2_g": "ln2_gb", "ln2_b": "ln2_gb", "b_qkv": "bqkv",
                "b_out": "bout", "b1": "b1", "b3": "b3", "b2": "b2"}[name]
        if flag in flags:
            vecs[name] = nc.declare_dram_parameter(name, [size], F32,
                                                   isOutput=False)
    out_p = nc.declare_dram_parameter("out", [T, D], F32, isOutput=True)

    KVB = (2 * KF + VF) // 2  # bf16 elems per rank (k bf16 + v fp8, packed)
    kv_in = nc.dram_tensor("kv_in", [KVB], BF16)
    kv_all = nc.dram_tensor("kv_all", [GROUP * KVB], BF16)

    groups = [list(range(g * GROUP, (g + 1) * GROUP))
              for g in range(N_CORES // GROUP)]

    def bcast_ap(param, width):
        return bass.AP(tensor=param.ap().tensor, offset=0,
                       ap=[[0, 128], [1, width]])

    from contextlib import ExitStack
    with tile.TileContext(nc) as tc, ExitStack() as stack:
        const = stack.enter_context(tc.tile_pool(name="const", bufs=1))
        ones1 = const.tile([1, 64], F32, tag="ones1")
        nc.vector.memset(ones1, 1.0)
        eps_t = const.tile([128, 1], F32, tag="eps")
        nc.vector.memset(eps_t, EPS)
        cosf = const.tile([128, NT, D], BF16, tag="cosf")
        sinm = const.tile([128, NT, D], BF16, tag="sinm")
        nc.sync.dma_start(
            out=cosf, in_=cos_p.ap().rearrange("(t p) d -> p t d", p=128))
        nc.sync.dma_start(
            out=sinm, in_=sin_p.ap().rearrange("(t p) d -> p t d", p=128))

        bc_tiles = {}
        for name in ("ln1_g", "ln1_b", "qn_g", "qn_b", "kn_g", "kn_b",
                     "ln2_g", "ln2_b", "b_out", "b2"):
            if name in vecs:
                t = const.tile([128, D], F32, tag=f"bc_{name}")
                nc.sync.dma_start(out=t, in_=bcast_ap(vecs[name], D))
                bc_tiles[name] = t
        if "b_qkv" in vecs:
            t = const.tile([128, D3], F32, tag="bc_bqkv")
            nc.sync.dma_start(out=t, in_=bcast_ap(vecs["b_qkv"], D3))
            bc_tiles["b_qkv"] = t
        for name in ("b1", "b3"):
            if name in vecs:
                t = const.tile([128, NH], F32, tag=f"col_{name}")
                ap = bass.AP(tensor=vecs[name].ap().tensor, offset=0,
                             ap=[[1, 128], [128, NH]])
                nc.sync.dma_start(out=t, in_=ap)
                bc_tiles[name] = t

        stat = stack.enter_context(tc.tile_pool(name="stat", bufs=4))
        xres = stack.enter_context(tc.tile_pool(name="xres", bufs=1))
        o1p = stack.enter_context(tc.tile_pool(name="o1p", bufs=1))

        x_N = [xres.tile([128, D], F32, tag=f"x{t}", name=f"x{t}")
               for t in range(NT)]
        out1_N = [o1p.tile([128, D], F32, tag=f"o1{t}", name=f"o1{t}")
                  for t in range(NT)]
        qkT_pool = stack.enter_context(tc.tile_pool(name="qkTp", bufs=1))
        # qA_T rows 0:64 = even-head q dims, rows 64:128 zeroed; qB_T the
        # mirror. Scores then run as full-128-contraction matmuls against
        # the unsplit K tiles (the other head's K rows hit zero Q rows).
        qA_T = qkT_pool.tile([128, ND, T], BF16, tag="qAT", name="qAT")
        qB_T = qkT_pool.tile([128, ND, T], BF16, tag="qBT", name="qBT")
        k_T = qkT_pool.tile([128, ND, T], BF16, tag="kT", name="kT")

        def ln_stats(src_tile):
            """rstd [128,1], negm_r [128,1] for LN over D free elems."""
            st = stat.tile([128, 2, 6], F32, tag="lnst")
            nc.vector.bn_stats(out=st[:, 0, :], in_=src_tile[:, 0:D // 2])
            nc.vector.bn_stats(out=st[:, 1, :], in_=src_tile[:, D // 2:D])
            mv = stat.tile([128, 2], F32, tag="lnmv")
            nc.vector.bn_aggr(out=mv, in_=st)
            rstd = stat.tile([128, 1], F32, tag="lnrstd")
            nc.scalar.activation(out=rstd, in_=mv[:, 1:2], func=AF.Sqrt,
                                 bias=eps_t, scale=1.0, alpha=0.0)
            nc.vector.reciprocal(out=rstd, in_=rstd)
            negmr = stat.tile([128, 1], F32, tag="lnnm")
            nc.vector.tensor_mul(out=negmr, in0=mv[:, 0:1], in1=rstd)
            nc.scalar.mul(out=negmr, in_=negmr, mul=-1.0)
            return rstd, negmr

        # ---- Phase A: load x, LN1 -> h (bf16), DMA-transpose -> h_T ------
        hTp = tc.tile_pool(name="hTp", bufs=1)
        hT_pool = hTp.__enter__()
        h_T = hT_pool.tile([128, ND, T], BF16, tag="hT", name="hT")
        with tc.tile_pool(name="hpool", bufs=2) as hpool:
            for t in range(NT):
                nc.sync.dma_start(out=x_N[t],
                                  in_=x_p.ap()[t * 128:(t + 1) * 128, :])
                h_N = hpool.tile([128, D], BF16, tag="hN")
                rstd, negmr = ln_stats(x_N[t])
                nc.scalar.activation(out=h_N, in_=x_N[t], func=AF.Identity,
                                     scale=rstd, bias=negmr, alpha=0.0)
                if "ln1_g" in bc_tiles:
                    nc.vector.tensor_mul(out=h_N, in0=h_N,
                                         in1=bc_tiles["ln1_g"])
                    nc.vector.tensor_add(out=h_N, in0=h_N,
                                         in1=bc_tiles["ln1_b"])
                nc.scalar.dma_start_transpose(
                    out=h_T[:, :, t * 128:(t + 1) * 128], in_=h_N)

        # ---- Phase B/C: QKV projection, K/V first, gathers early ---------
        # wqkv_t storage order: [k0, k1, v0, v1, q0, q1]
        qknp_cm = tc.tile_pool(name="qknp", bufs=1)
        qknp = qknp_cm.__enter__()
        q_N = [qknp.tile([128, D], BF16, tag=f"qN{t}", name=f"qN{t}")
               for t in range(NT)]
        k_N = [qknp.tile([128, D], BF16, tag=f"kN{t}", name=f"kN{t}")
               for t in range(NT)]
        vpp = tc.tile_pool(name="vpp", bufs=1)
        vp_pool = vpp.__enter__()
        v_flat = vp_pool.tile([128, NT, VW], FP8, tag="vpad", name="vpad")
        v_pad = v_flat.rearrange("p t (h f) -> p t h f", h=H)
        nc.vector.memset(v_pad[:, :, :, HD:HD + 1], 1.0)

        wq_cm = tc.tile_pool(name="wq", bufs=2)
        wq_pool = wq_cm.__enter__()
        mmps_cm = tc.tile_pool(name="mmps", bufs=2, space="PSUM")
        mmps = mmps_cm.__enter__()

        def qkv_chunk(ci):
            """ci: storage index into wqkv_t [k0,k1,v0,v1,q0,q1]."""
            w = wq_pool.tile([128, ND, 512], BF16, tag="wqt")
            nc.sync.dma_start(out=w, in_=wqkv_p.ap()[ci])
            ps = [mmps.tile([128, 512], F32, tag=f"qkvps{t}",
                            name=f"qkvps_{ci}_{t}") for t in range(NT)]
            for d in range(ND):
                for t in range(NT):
                    nc.tensor.matmul(
                        ps[t], h_T[:, d, t * 128:(t + 1) * 128], w[:, d, :],
                        start=(d == 0), stop=(d == ND - 1))
            # logical chunk: 0,1=q; 2,3=k; 4,5=v
            ch = [2, 3, 4, 5, 0, 1][ci]
            for t in range(NT):
                if ch < 2:
                    dst = q_N[t][:, (ch % 2) * 512:(ch % 2) * 512 + 512]
                elif ch < 4:
                    dst = k_N[t][:, (ch % 2) * 512:(ch % 2) * 512 + 512]
                else:
                    h0 = (ch - 4) * 8
                    dst = v_pad[:, t, h0:h0 + 8, 0:HD]
                    if "b_qkv" in bc_tiles:
                        nc.vector.tensor_add(
                            out=dst,
                            in0=bc_tiles["b_qkv"][:, ch * 512:(ch + 1) * 512]
                            .rearrange("p (h f) -> p h f", h=8),
                            in1=ps[t].rearrange("p (h f) -> p h f", h=8))
                    else:
                        nc.vector.tensor_copy(
                            out=dst,
                            in_=ps[t].rearrange("p (h f) -> p h f", h=8))
                    continue
                if "b_qkv" in bc_tiles:
                    nc.vector.tensor_add(
                        out=dst,
                        in0=bc_tiles["b_qkv"][:, ch * 512:(ch + 1) * 512],
                        in1=ps[t])
                else:
                    nc.vector.tensor_copy(out=dst, in_=ps[t])

        def qknorm_rope(src_N, dst_Ts, gname, t):
            """qk-norm + rope on [128, D] bf16, DMA-transpose into dst_Ts."""
            rstd, negmr = ln_stats(src_N)
            nrm = rope_pool.tile([128, D], BF16, tag="nrm")
            nc.scalar.activation(out=nrm, in_=src_N, func=AF.Identity,
                                 scale=rstd, bias=negmr, alpha=0.0)
            if f"{gname}_g" in bc_tiles:
                nc.vector.tensor_mul(out=nrm, in0=nrm,
                                     in1=bc_tiles[f"{gname}_g"])
                nc.vector.tensor_add(out=nrm, in0=nrm,
                                     in1=bc_tiles[f"{gname}_b"])
            nrm3 = nrm.rearrange("p (h f) -> p h f", h=H)
            sw = rope_pool.tile([128, H, HD], BF16, tag="sw")
            nc.vector.tensor_copy(out=sw[:, :, 0:32], in_=nrm3[:, :, 32:64])
            nc.vector.tensor_copy(out=sw[:, :, 32:64], in_=nrm3[:, :, 0:32])
            swf = sw.rearrange("p h f -> p (h f)")
            rp = rope_pool.tile([128, D], BF16, tag="rp")
            nc.vector.tensor_mul(out=rp, in0=nrm, in1=cosf[:, t, :])
            nc.vector.tensor_mul(out=swf, in0=swf, in1=sinm[:, t, :])
            nc.vector.tensor_add(out=rp, in0=rp, in1=swf)
            for dst_T in dst_Ts:
                nc.scalar.dma_start_transpose(
                    out=dst_T[:, :, t * 128:(t + 1) * 128], in_=rp)

        rope_cm = tc.tile_pool(name="ropep", bufs=2)
        rope_pool = rope_cm.__enter__()

        # K chunks -> k norm/rope -> bounce; V chunks -> bounce; one gather
        qkv_chunk(0)
        qkv_chunk(1)
        for t in range(NT):
            qknorm_rope(k_N[t], [k_T], "kn", t)
        k_dst = bass.AP(tensor=kv_in.ap().tensor, offset=0,
                        ap=[[512, 128], [128 * 512, ND], [1, 512]])
        nc.scalar.dma_start(out=k_dst, in_=k_T)

        qkv_chunk(2)
        qkv_chunk(3)
        v_dst = bass.AP(tensor=kv_in.ap().tensor, offset=KF,
                        ap=[[VW // 2, 128], [128 * VW // 2, NT], [1, VW // 2]])
        nc.scalar.dma_start(out=v_dst, in_=v_flat.bitcast(BF16))
        nc.gpsimd.collective_compute(
            "AllGather", mybir.AluOpType.bypass, replica_groups=groups,
            ins=[kv_in.ap().opt()], outs=[kv_all.ap().opt()])

        # Q chunks + q norm/rope (overlaps the gather)
        qkv_chunk(4)
        qkv_chunk(5)
        for t in range(NT):
            qknorm_rope(q_N[t], [qA_T, qB_T], "qn", t)
            sl = slice(t * 128, (t + 1) * 128)
            nc.vector.memset(qA_T[64:128, :, sl], 0.0)
            nc.vector.memset(qB_T[0:64, :, sl], 0.0)

        rope_cm.__exit__(None, None, None)
        mmps_cm.__exit__(None, None, None)
        wq_cm.__exit__(None, None, None)
        vpp.__exit__(None, None, None)
        qknp_cm.__exit__(None, None, None)
        hTp.__exit__(None, None, None)

        # ---- Phase E: attention ------------------------------------------
        h2Tp = stack.enter_context(tc.tile_pool(name="h2Tp", bufs=1))
        h2_T = h2Tp.tile([128, ND, T], BF16, tag="h2T", name="h2T")
        stackp_cm = tc.tile_pool(name="stackp", bufs=1)
        stackp = stackp_cm.__enter__()
        stacked = stackp.tile([128, ND, T], BF16, tag="stk", name="stk")

        kvrem_cm = tc.tile_pool(name="kvrem", bufs=1)
        kvrem = kvrem_cm.__enter__()
        k_r = []
        v_r = []
        for rc in range(GROUP):
            kt_ = kvrem.tile([128, ND, T], BF16, tag=f"kr{rc}", name=f"kr{rc}")
            src = bass.AP(tensor=kv_all.ap().tensor, offset=rc * KVB,
                          ap=[[512, 128], [128 * 512, ND], [1, 512]])
            nc.scalar.dma_start(out=kt_, in_=src)
            k_r.append(kt_)
            vt_ = kvrem.tile([128, NT, VW], FP8, tag=f"vr{rc}",
                             name=f"vr{rc}")
            src = bass.AP(tensor=kv_all.ap().tensor,
                          offset=rc * KVB + KF,
                          ap=[[VW // 2, 128], [128 * VW // 2, NT],
                              [1, VW // 2]])
            nc.scalar.dma_start(out=vt_.bitcast(BF16), in_=src)
            v_r.append(vt_.rearrange("p t (h f) -> p t h f", h=H))

        with (
            tc.tile_pool(name="scps", bufs=2, space="PSUM") as scps,
            tc.tile_pool(name="pvps", bufs=1, space="PSUM") as pvps,
            tc.tile_pool(name="bcps", bufs=1, space="PSUM") as bcps,
            tc.tile_pool(name="prb", bufs=3) as prb,
            tc.tile_pool(name="accp", bufs=2) as accp,
            tc.tile_pool(name="tbp", bufs=2) as tbp,
        ):
            for d in range(ND):
                pvA = pvps.tile([65, T], F32, tag="pvA", name=f"pvA{d}")
                pvB = pvps.tile([65, T], F32, tag="pvB", name=f"pvB{d}")
                for half, pv in (("A", pvA), ("B", pvB)):
                    q_h = qA_T if half == "A" else qB_T
                    hidx = 2 * d if half == "A" else 2 * d + 1
                    # software pipeline: scores+exp for pair p+1 issued
                    # before the PV of pair p (keeps PE and ACT busy)
                    prs = [None] * NPAIR

                    def scores(p):
                        rc, pl = divmod(p, NT // 2)
                        pr = prb.tile([128, 2, T], FP8, tag="prp",
                                      name=f"pr_{d}_{half}_{p}")
                        ps = scps.tile([128, 2, T], F32, tag="scp",
                                       name=f"sc_{d}_{half}_{p}")
                        for j in range(2):
                            kt = pl * 2 + j
                            sl = slice(kt * 128, (kt + 1) * 128)
                            nc.tensor.matmul(
                                ps[:, j, :],
                                k_r[rc][:, d, sl],
                                q_h[:, d, :],
                                start=True, stop=True)
                        nc.scalar.activation(
                            out=pr, in_=ps,
                            func=AF.Exp, scale=1.0 / np.sqrt(HD),
                            alpha=0.0)
                        prs[p] = pr

                    def pv_acc(p):
                        rc, pl = divmod(p, NT // 2)
                        vsl = v_r[rc][:, pl * 2:pl * 2 + 2, hidx, :]
                        nc.tensor.matmul(
                            pv, vsl, prs[p],
                            start=(p == 0), stop=(p == NPAIR - 1),
                            perf_mode=DR)

                    scores(0)
                    for p in range(NPAIR):
                        if p + 1 < NPAIR:
                            scores(p + 1)
                        pv_acc(p)

                # denominators: reciprocal of the ones-row, K=1 matmul
                # broadcast across 64 partitions, multiply
                for half, pv in (("A", pvA), ("B", pvB)):
                    acc = accp.tile([65, T], F32, tag="acc")
                    nc.vector.tensor_copy(out=acc.bitcast(F32R), in_=pv)
                    rrow = tbp.tile([1, T], F32, tag="rrow")
                    with nc.allow_low_precision(reason="f32r bits are f32"):
                        nc.vector.reciprocal(out=rrow.bitcast(F32R),
                                             in_=acc[64:65, :])
                    bc = bcps.tile([64, T], F32, tag="bc")
                    nc.tensor.matmul(bc, ones1.bitcast(F32R),
                                     rrow.bitcast(F32R),
                                     start=True, stop=True)
                    if half == "A":
                        nc.vector.tensor_mul(out=stacked[0:64, d, :],
                                             in0=acc[0:64, :], in1=bc)
                    else:
                        tmpB = tbp.tile([64, T], BF16, tag="tmpB")
                        nc.vector.tensor_mul(out=tmpB, in0=acc[0:64, :],
                                             in1=bc)
                        nc.sync.dma_start(out=stacked[64:128, d, :], in_=tmpB)

        kvrem_cm.__exit__(None, None, None)

        # ---- Phase F: out projection + residual + LN2, pipelined per t ---
        with (
            tc.tile_pool(name="wo", bufs=1) as wo,
            tc.tile_pool(name="ops", bufs=2, space="PSUM") as ops,
            tc.tile_pool(name="h2pool", bufs=2) as h2pool,
        ):
            wot = []
            for ch in range(2):
                w = wo.tile([128, ND, 512], BF16, tag=f"wot{ch}")
                nc.sync.dma_start(out=w, in_=wout_p.ap()[ch])
                wot.append(w)
            for t in range(NT):
                ps = [ops.tile([128, 512], F32, tag=f"ops{ch}",
                               name=f"ops_{ch}_{t}") for ch in range(2)]
                for d in range(ND):
                    for ch in range(2):
                        nc.tensor.matmul(
                            ps[ch], stacked[:, d, t * 128:(t + 1) * 128],
                            wot[ch][:, d, :],
                            start=(d == 0), stop=(d == ND - 1))
                for ch in range(2):
                    sl = slice(ch * 512, (ch + 1) * 512)
                    nc.vector.tensor_add(out=out1_N[t][:, sl],
                                         in0=x_N[t][:, sl], in1=ps[ch])
                    if "b_out" in bc_tiles:
                        nc.vector.tensor_add(out=out1_N[t][:, sl],
                                             in0=out1_N[t][:, sl],
                                             in1=bc_tiles["b_out"][:, sl])
                h2_N = h2pool.tile([128, D], BF16, tag="h2N")
                rstd, negmr = ln_stats(out1_N[t])
                nc.scalar.activation(out=h2_N, in_=out1_N[t], func=AF.Identity,
                                     scale=rstd, bias=negmr, alpha=0.0)
                if "ln2_g" in bc_tiles:
                    nc.vector.tensor_mul(out=h2_N, in0=h2_N,
                                         in1=bc_tiles["ln2_g"])
                    nc.vector.tensor_add(out=h2_N, in0=h2_N,
                                         in1=bc_tiles["ln2_b"])
                nc.scalar.dma_start_transpose(
                    out=h2_T[:, :, t * 128:(t + 1) * 128], in_=h2_N)

        stackp_cm.__exit__(None, None, None)

        # ---- Phase H: FFN1/FFN3 -> prod_T --------------------------------
        prp = stack.enter_context(tc.tile_pool(name="prp", bufs=1))
        prod_T = [prp.tile([128, T], BF16, tag=f"pr{h}", name=f"pr{h}")
                  for h in range(NH)]
        with (
            tc.tile_pool(name="wf", bufs=3) as wf,
            tc.tile_pool(name="ffps", bufs=2, space="PSUM") as ffps,
            tc.tile_pool(name="s1p", bufs=2) as s1p,
        ):
            for ht in range(NH):
                w1sb = wf.tile([128, ND, 128], BF16, tag="w1sb")
                w3sb = wf.tile([128, ND, 128], BF16, tag="w3sb")
                nc.sync.dma_start(out=w1sb, in_=w1_p.ap()[ht])
                nc.sync.dma_start(out=w3sb, in_=w3_p.ap()[ht])
                ps1 = ffps.tile([128, T], F32, tag="ps1")
                ps3 = ffps.tile([128, T], F32, tag="ps3")
                for d in range(ND):
                    nc.tensor.matmul(ps1, w1sb[:, d, :], h2_T[:, d, :],
                                     start=(d == 0), stop=(d == ND - 1))
                for d in range(ND):
                    nc.tensor.matmul(ps3, w3sb[:, d, :], h2_T[:, d, :],
                                     start=(d == 0), stop=(d == ND - 1))
                s1 = s1p.tile([128, T], BF16, tag="s1")
                b1arg = bc_tiles["b1"][:, ht:ht + 1] if "b1" in bc_tiles else 0.0
                nc.scalar.activation(out=s1, in_=ps1, func=AF.Silu,
                                     bias=b1arg, scale=1.0, alpha=0.0)
                t3 = s1p.tile([128, T], BF16, tag="t3")
                if "b3" in bc_tiles:
                    nc.vector.tensor_scalar_add(
                        out=t3, in0=ps3, scalar1=bc_tiles["b3"][:, ht:ht + 1])
                else:
                    nc.vector.tensor_copy(out=t3, in_=ps3)
                nc.vector.tensor_mul(out=prod_T[ht], in0=s1, in1=t3)

        # ---- Phase I: FFN2 + residual + store ----------------------------
        with (
            tc.tile_pool(name="w2p", bufs=2) as w2p,
            tc.tile_pool(name="f2ps", bufs=1, space="PSUM") as f2ps,
        ):
            ps = [[f2ps.tile([128, 512], F32, tag=f"f2_{t}_{ch}",
                             name=f"f2_{t}_{ch}") for ch in range(2)]
                  for t in range(NT)]
            for hg in range(NH // 8):
                w2g = w2p.tile([128, 8, D], BF16, tag="w2g")
                nc.sync.dma_start(out=w2g, in_=w2_p.ap()[hg])
                for hi in range(8):
                    ht = hg * 8 + hi
                    for t in range(NT):
                        for ch in range(2):
                            nc.tensor.matmul(
                                ps[t][ch],
                                prod_T[ht][:, t * 128:(t + 1) * 128],
                                w2g[:, hi, ch * 512:(ch + 1) * 512],
                                start=(ht == 0), stop=(ht == NH - 1))
            for t in range(NT):
                for ch in range(2):
                    sl = slice(ch * 512, (ch + 1) * 512)
                    nc.vector.tensor_add(out=out1_N[t][:, sl],
                                         in0=out1_N[t][:, sl], in1=ps[t][ch])
                    if "b2" in bc_tiles:
                        nc.vector.tensor_add(out=out1_N[t][:, sl],
                                             in0=out1_N[t][:, sl],
                                             in1=bc_tiles["b2"][:, sl])
                nc.sync.dma_start(out=out_p.ap()[t * 128:(t + 1) * 128, :],
                                  in_=out1_N[t])

    _split_all_waits(nc)
    return nc


# ---------------------------------------------------------------------------
# Host wrapper
# ---------------------------------------------------------------------------

_CACHE = {}
_PREP_CACHE = {}


def _prep_inputs(x, rope_cos, rope_sin, w_qkv, b_qkv, w_out, b_out,
                 qn_g, qn_b, kn_g, kn_b, ln1_g, ln1_b, ln2_g, ln2_b,
                 w1, b1, w2, b2, w3, b3):
    B, S, D = x.shape
    H, HD, FFN = 16, 64, 4096
    T = B * S // N_CORES
    ND, NH = D // 128, FFN // 128

    wkey = (id(w_qkv), id(w_out), id(w1), id(w2), id(w3),
            id(rope_cos), id(rope_sin))
    if wkey in _PREP_CACHE:
        shared, flags = _PREP_CACHE[wkey]
    else:
        flags = set()
        if not (np.all(ln1_g == 1) and np.all(ln1_b == 0)):
            flags.add("ln1_gb")
        if not (np.all(qn_g == 1) and np.all(qn_b == 0)):
            flags.add("qn_gb")
        if not (np.all(kn_g == 1) and np.all(kn_b == 0)):
            flags.add("kn_gb")
        if not (np.all(ln2_g == 1) and np.all(ln2_b == 0)):
            flags.add("ln2_gb")
        if np.any(b_qkv != 0):
            flags.add("bqkv")
        if np.any(b_out != 0):
            flags.add("bout")
        if np.any(b1 != 0):
            flags.add("b1")
        if np.any(b2 != 0):
            flags.add("b2")
        if np.any(b3 != 0):
            flags.add("b3")
        flags = frozenset(flags)

        bf = ml_dtypes.bfloat16
        # rope tables [S, D]: cos tiled over heads; sin with sign folded
        cosfull = np.tile(rope_cos, (1, H)).astype(bf)
        sinmod_half = np.concatenate(
            [-rope_sin[:, :HD // 2], rope_sin[:, HD // 2:]], axis=1)
        sinmod = np.tile(sinmod_half, (1, H)).astype(bf)

        wqkvT = np.ascontiguousarray(w_qkv.T)           # [D, 3D]
        # wqkv_t [6, 128, ND, 512]: storage order [k0,k1,v0,v1,q0,q1];
        # [ci, p, d, c] = wqkvT[d*128+p, ch*512+c]
        ch_order = [2, 3, 4, 5, 0, 1]
        wq4 = wqkvT.reshape(ND, 128, 6, 512)            # [d, p, ch, c]
        wqkv_t = np.ascontiguousarray(
            wq4.transpose(2, 1, 0, 3)[ch_order]).astype(bf)
        woutT = np.ascontiguousarray(w_out.T)           # [D, D]
        wo4 = woutT.reshape(ND, 128, 2, 512)
        wout_t = np.ascontiguousarray(wo4.transpose(2, 1, 0, 3)).astype(bf)
        # w1_t [NH, 128, ND, 128]: [ht, p, d, c] = w1[ht*128+c, d*128+p]
        w1r = w1.reshape(NH, 128, ND, 128)              # [ht, c, d, p]
        w1_t = np.ascontiguousarray(w1r.transpose(0, 3, 2, 1)).astype(bf)
        w3r = w3.reshape(NH, 128, ND, 128)
        w3_t = np.ascontiguousarray(w3r.transpose(0, 3, 2, 1)).astype(bf)
        # w2_t [NH//8, 128, 8, D]: [hg, p, hi, c] = w2[c, (hg*8+hi)*128+p]
        w2r = w2.reshape(D, NH // 8, 8, 128)            # [c, hg, hi, p]
        w2_t = np.ascontiguousarray(w2r.transpose(1, 3, 2, 0)).astype(bf)

        shared = {
            "wqkv_t": wqkv_t, "wout_t": wout_t,
            "w1_t": w1_t, "w3_t": w3_t, "w2_t": w2_t,
            "cosfull": cosfull, "sinmod": sinmod,
            # keep refs so ids stay unique
            "_refs": (w_qkv, w_out, w1, w2, w3, rope_cos, rope_sin),
        }
        opt = {"ln1_gb": [("ln1_g", ln1_g), ("ln1_b", ln1_b)],
               "qn_gb": [("qn_g", qn_g), ("qn_b", qn_b)],
               "kn_gb": [("kn_g", kn_g), ("kn_b", kn_b)],
               "ln2_gb": [("ln2_g", ln2_g), ("ln2_b", ln2_b)],
               "bqkv": [("b_qkv", b_qkv)], "bout": [("b_out", b_out)],
               "b1": [("b1", b1)], "b2": [("b2", b2)], "b3": [("b3", b3)]}
        for fl, items in opt.items():
            if fl in flags:
                for name, arr in items:
                    shared[name] = np.ascontiguousarray(arr).astype(np.float32)
        _PREP_CACHE[wkey] = (shared, flags)

    xf = np.ascontiguousarray(x.reshape(B * S, D)).astype(np.float32)
    in_maps = []
    for c in range(N_CORES):
        t0 = c * T
        m = {k: v for k, v in shared.items() if k != "_refs"}
        m["x"] = xf[t0:t0 + T]
        m["cosfull"] = shared["cosfull"][t0 % S:t0 % S + T]
        m["sinmod"] = shared["sinmod"][t0 % S:t0 % S + T]
        in_maps.append(m)
    return in_maps, flags, T, D


def kernel(**inputs):
    x = inputs["x"]
    B, S, D = x.shape
    in_maps, flags, T, _ = _prep_inputs(**inputs)

    key = (T, D, flags)
    if key not in _CACHE:
        _CACHE[key] = build_nc(T=T, D=D, flags=flags)
    nc = _CACHE[key]

    res = run_bass_kernel_spmd(nc, in_maps, core_ids=list(range(N_CORES)))
    out = np.empty((B * S, D), np.float32)
    for c in range(N_CORES):
        out[c * T:(c + 1) * T] = res.results[c]["out"]
    return out.reshape(B, S, D)
